# revision 21
# baseline (speedup 1.0000x reference)
"""Trainium2 Bass kernel for nn_GSAttention (spatial-reduction attention).

Strategy (v2)
-------------
* Queries sharded 512/core x 8 cores; B=2 kept on-core (1024 query rows).
* Only the first 96 reduced KV tokens are visible (max vis = 94), so the
  whole KV path (conv + LN + KV proj) runs on 96 tokens, replicated.
* All big matmuls in fp8e4m3 DoubleRow mode (4x bf16 throughput):
  q-proj, conv, k-proj, v-proj, out-proj, and the additive causal mask
  (one-hot matmul, values 0/-240 exact in fp8).  QK / AV / softmax-denom /
  denom-broadcast matmuls in bf16.  Weights are host-scaled x64 to clear
  the fp8 subnormal range; the scale is folded back into the RoPE tables
  (q/k paths) and the final output-evacuation scale (v/proj paths).
* Mask as matmul: z += tri^T @ onehot(vis) accumulated into the QK PSUM
  group; exp then masks to ~1e-13 naturally.
* Softmax denominators: per (head,batch) pair j in a group of 4, a
  [96,128] one-hot-column stationary places den_j at PSUM partition 32j;
  one DVE reciprocal per group; per-pair broadcast matmul (ones row at
  partition 32j x rec row) rebuilds [64,512] 1/den; the division fuses
  the PSUM->SBUF evacuation of AV into the DR-packed fp8 att2 tile.
* Out-proj in [c_out, q] orientation so the projection bias and the
  1/4096 fp8-scale fold into the Activation-engine PSUM evacuation.
  Host transposes the returned [768, 1024] tiles.
* One DMA per tensor, host pre-packed in device layouts (fp8/bf16):
  ~8.5 MB total HBM traffic per core.
"""

import os
import sys

for _p in ("/opt/trn_rl_repo", "/root/.axon_site/_ro/trn_rl_repo"):
    if os.path.isdir(_p) and _p not in sys.path:
        sys.path.insert(0, _p)

from contextlib import ExitStack

import numpy as np
import ml_dtypes

import types as _types
if "antenv.axon_hooks" not in sys.modules:
    _axh = _types.ModuleType("antenv.axon_hooks")
    _axh.get_axon_ntff_profile_hook = lambda: None
    sys.modules["antenv.axon_hooks"] = _axh

import concourse.bacc as bacc
import concourse.mybir as mybir
from concourse.tile import TileContext
from concourse.bass_utils import run_bass_kernel_spmd

F32 = mybir.dt.float32
BF16 = mybir.dt.bfloat16
FP8 = mybir.dt.float8e4
AF = mybir.ActivationFunctionType
ALU = mybir.AluOpType
DR = mybir.MatmulPerfMode.DoubleRow

NP_BF16 = ml_dtypes.bfloat16
NP_FP8 = ml_dtypes.float8_e4m3

# Problem constants.
N_CORES = 8
B = 2
N = 4096
C = 768
HEADS = 12
HD = 64
SR = 2
H = W = 64
NQ = 512            # queries per core per batch
NQT = B * NQ        # query rows per core
M = 96              # padded visible reduced tokens (real max vis = 94)
M2 = B * M
KC = C * SR * SR    # 3072 conv contraction
SCALE = 1.0 / 8.0
NEG = -240.0        # fp8e4m3 max; exp(scale*-240) ~ 1e-13
NCH = C // 128      # 6 feature chunks

# fp8 quantization of any data path costs 2.6-4% max-rel-err (threshold 2e-2),
# so data matmuls run bf16; only the exact-valued mask matmul uses fp8 DR.
FP8_PATHS = set()
WS = 64.0 if FP8_PATHS else 1.0


def build_program():
    nc = bacc.Bacc("TRN2", target_bir_lowering=False, debug=False,
                   num_devices=N_CORES)

    def par(name, shape, dt, out=False):
        return nc.declare_dram_parameter(name, list(shape), dt, isOutput=out)

    # host-packed inputs (one DMA each)
    DT_Q = FP8 if "q" in FP8_PATHS else BF16
    DT_CONV = FP8 if "conv" in FP8_PATHS else BF16
    DT_KV = FP8 if "kv" in FP8_PATHS else BF16
    DT_ATT = FP8 if "att" in FP8_PATHS else BF16
    xT_dr = par("xT_dr", (128, 3, 2, NQT), DT_Q)
    wq_dr = par("wq_dr", (128, 3, 2, C), DT_Q)
    wk_dr = par("wk_dr", (128, 3, 2, C), DT_KV)
    wv_dr = par("wv_dr", (128, 3, 2, C), DT_KV)
    srw_dr = par("srw_dr", (128, 12, 2, C), DT_CONV)
    xi_dr = par("xi_dr", (128, 12, 2, M2), DT_CONV)
    pj_dr = par("pj_dr", (128, 3, 2, C), DT_ATT)
    cq = par("cq", (128, NQT), BF16)
    sq = par("sq", (128, NQT), BF16)
    ck = par("ck", (128, M2), BF16)
    sk = par("sk", (128, M2), BF16)
    tri_dr = par("tri_dr", (48, 2, M), FP8)
    oh_dr = par("oh_dr", (48, 2, NQ), FP8)
    denoh = par("denoh", (M, 512), BF16)
    sel2 = par("sel2", (128, 128), BF16)
    onesb = par("onesb", (128, 65), BF16)
    biases = par("biases", (128, 4 * NCH), F32)   # srb | lng | lnb | pbias
    Y = par("y", (C, NQT), BF16, out=True)

    with TileContext(nc) as tc, ExitStack() as st:
        pers = st.enter_context(tc.tile_pool(name="pers", bufs=1))

        # ---- persistent tiles ----
        cq_t = pers.tile([128, NQT], BF16, tag="cq", name="cq")
        sq_t = pers.tile([128, NQT], BF16, tag="sq", name="sq")
        ck_t = pers.tile([128, M2], BF16, tag="ck", name="ck")
        sk_t = pers.tile([128, M2], BF16, tag="sk", name="sk")
        tri_t = pers.tile([48, 2, M], FP8, tag="tri", name="tri")
        oh_t = pers.tile([48, 2, NQ], FP8, tag="oh", name="oh")
        denoh_t = pers.tile([M, 512], BF16, tag="denoh", name="denoh")
        sel2_t = pers.tile([128, 128], BF16, tag="sel2", name="sel2")
        onesb_t = pers.tile([128, 65], BF16, tag="onesb", name="onesb")
        bias_t = pers.tile([128, 4 * NCH], F32, tag="bias", name="bias")
        xT_t = pers.tile([128, 3, 2, NQT], DT_Q, tag="xT", name="xT")
        wq_t = pers.tile([128, 3, 2, C], DT_Q, tag="wq", name="wq")
        wk_t = pers.tile([128, 3, 2, C], DT_KV, tag="wk", name="wk")
        wv_t = pers.tile([128, 3, 2, C], DT_KV, tag="wv", name="wv")
        srw_t = pers.tile([128, 12, 2, C], DT_CONV, tag="srw", name="srw")
        xi_t = pers.tile([128, 12, 2, M2], DT_CONV, tag="xi", name="xi")
        pj_t = pers.tile([128, 3, 2, C], DT_ATT, tag="pj", name="pj")
        rotq = [pers.tile([128, NQT], BF16, tag=f"rotq{i}", name=f"rotq{i}")
                for i in range(NCH)]
        rotk = [pers.tile([128, M2], BF16, tag=f"rotk{i}", name=f"rotk{i}")
                for i in range(NCH)]
        xln2 = pers.tile([128, 3, 2, M2], DT_KV, tag="xln2", name="xln2")
        vaug = [pers.tile([M, C], BF16, tag=f"vaug{b}", name=f"vaug{b}")
                for b in range(B)]
        att2 = pers.tile([128, 3, 2, NQT], DT_ATT, tag="att2", name="att2")

        # DMA priority order: q path first, then KV path, proj last.
        nc.sync.dma_start(out=bias_t[:], in_=biases[:])
        nc.sync.dma_start(out=xT_t[:, 0, :, :], in_=xT_dr[:, 0, :, :])
        nc.sync.dma_start(out=wq_t[:, 0, :, :], in_=wq_dr[:, 0, :, :])
        nc.sync.dma_start(out=xi_t[:], in_=xi_dr[:])
        for g in range(2):
            nc.sync.dma_start(out=srw_t[:, 6 * g:6 * (g + 1), :, :],
                              in_=srw_dr[:, 6 * g:6 * (g + 1), :, :])
        for j in range(1, 3):
            nc.sync.dma_start(out=xT_t[:, j, :, :], in_=xT_dr[:, j, :, :])
            nc.sync.dma_start(out=wq_t[:, j, :, :], in_=wq_dr[:, j, :, :])
        nc.sync.dma_start(out=cq_t[:], in_=cq[:])
        nc.sync.dma_start(out=sq_t[:], in_=sq[:])
        nc.sync.dma_start(out=wk_t[:], in_=wk_dr[:])
        nc.sync.dma_start(out=wv_t[:], in_=wv_dr[:])
        nc.sync.dma_start(out=ck_t[:], in_=ck[:])
        nc.sync.dma_start(out=sk_t[:], in_=sk[:])
        nc.sync.dma_start(out=tri_t[:], in_=tri_dr[:])
        nc.sync.dma_start(out=oh_t[:], in_=oh_dr[:])
        nc.sync.dma_start(out=denoh_t[:], in_=denoh[:])
        nc.sync.dma_start(out=sel2_t[:], in_=sel2[:])
        nc.sync.dma_start(out=onesb_t[:], in_=onesb[:])
        nc.sync.dma_start(out=pj_t[:], in_=pj_dr[:])

        def mm_chain(out_ap, stat_fn, mov_fn, nblk, fp8, tp=None):
            """Accumulate out += stat_j^T @ mov_j over nblk k-blocks."""
            for j in range(nblk):
                if fp8:
                    nc.tensor.matmul(out_ap, stat_fn(j, None), mov_fn(j, None),
                                     start=(j == 0), stop=(j == nblk - 1),
                                     perf_mode=DR, tile_position=tp)
                else:
                    for t in range(2):
                        nc.tensor.matmul(out_ap, stat_fn(j, t), mov_fn(j, t),
                                         start=(j == 0 and t == 0),
                                         stop=(j == nblk - 1 and t == 1),
                                         tile_position=tp)

        def dr_slice(tile, j, t, cols):
            return tile[:, j, :, cols] if t is None else tile[:, j, t, cols]

        srb_c = lambda o: bias_t[:, o:o + 1]
        lng_c = lambda o: bias_t[:, NCH + o:NCH + o + 1]
        lnb_c = lambda o: bias_t[:, 2 * NCH + o:2 * NCH + o + 1]
        pb_c = lambda o: bias_t[:, 3 * NCH + o:3 * NCH + o + 1]

        # ========== Phase 1: q-projection + conv (fp8 DR) ==========
        with tc.tile_pool(name="p1", bufs=1) as p1, \
             tc.tile_pool(name="p1q", bufs=2) as p1q, \
             tc.tile_pool(name="ps1", bufs=1, space="PSUM") as ps1:
            q_sb = [p1.tile([128, NQT], BF16, tag=f"qsb{i}", name=f"qsb{i}")
                    for i in range(NCH)]

            def q_block(cc, half):
                ns = slice(half * NQ, (half + 1) * NQ)
                q_ps = ps1.tile([128, NQ], F32, tag=f"qp{(2 * cc + half) % 2}",
                                name=f"qp{cc}{half}")
                mm_chain(q_ps[:],
                         lambda j, t: dr_slice(wq_t, j, t,
                                               slice(cc * 128, (cc + 1) * 128)),
                         lambda j, t: dr_slice(xT_t, j, t, ns),
                         3, "q" in FP8_PATHS)
                nc.scalar.activation(q_sb[cc][:, ns], q_ps[:], AF.Identity)

            xr_ps = [ps1.tile([128, M2], F32, tag=f"xr{o}", name=f"xr{o}")
                     for o in range(NCH)]

            conv_fp8 = "conv" in FP8_PATHS

            def conv_col(o, jlo, jhi):
                for j2 in range(jlo, jhi):
                    if conv_fp8:
                        nc.tensor.matmul(
                            xr_ps[o][:], srw_t[:, j2, :, o * 128:(o + 1) * 128],
                            xi_t[:, j2, :, :], start=(j2 == 0), stop=(j2 == 11),
                            perf_mode=DR)
                    else:
                        for t in range(2):
                            nc.tensor.matmul(
                                xr_ps[o][:],
                                srw_t[:, j2, t, o * 128:(o + 1) * 128],
                                xi_t[:, j2, t, :],
                                start=(j2 == 0 and t == 0),
                                stop=(j2 == 11 and t == 1))

            # interleave: 12 q blocks with 12 conv half-columns
            for step in range(12):
                q_block(step // 2, step % 2)
                o, hcol = step % 6, step // 6
                conv_col(o, 6 * hcol, 6 * (hcol + 1))

            # ---- RoPE on q (bf16 2x on DVE; half-ops split with Pool) ----
            for o in range(3):
                t1 = p1q.tile([128, NQT], BF16, tag="rt1", name="rt1")
                t2 = p1q.tile([128, NQT], BF16, tag="rt2", name="rt2")
                nc.vector.tensor_mul(t1[:], q_sb[o][:], cq_t[:])
                nc.vector.tensor_mul(t2[:], q_sb[o + 3][:], sq_t[:])
                nc.vector.tensor_sub(rotq[o][:], t1[:], t2[:])
                t3 = p1q.tile([128, NQT], BF16, tag="rt3", name="rt3")
                t4 = p1q.tile([128, NQT], BF16, tag="rt4", name="rt4")
                nc.vector.tensor_mul(t3[:], q_sb[o + 3][:], cq_t[:])
                nc.vector.tensor_mul(t4[:], q_sb[o][:], sq_t[:])
                nc.vector.tensor_add(rotq[o + 3][:], t3[:], t4[:])

            # ========== Phase 2: LN + K/V ==========
            xr_sb = [p1.tile([128, M2], BF16, tag=f"xs{o}", name=f"xs{o}")
                     for o in range(NCH)]
            for o in range(NCH):
                nc.scalar.activation(xr_sb[o][:], xr_ps[o][:],
                                     AF.Identity, bias=srb_c(o))

            ones_ln = onesb_t[:, 0:1]
            sum_ps = ps1.tile([1, M2], F32, tag="xr0", name="sum")
            for o in range(NCH):
                nc.tensor.matmul(sum_ps[:], ones_ln, xr_sb[o][:],
                                 start=(o == 0), stop=(o == NCH - 1))
            ssq_ps = ps1.tile([1, M2], F32, tag="xr1", name="ssq")
            for o in range(NCH):
                sqt = p1q.tile([128, M2], BF16, tag="sqt", name="sqt")
                nc.gpsimd.tensor_mul(sqt[:], xr_sb[o][:], xr_sb[o][:])
                nc.tensor.matmul(ssq_ps[:], ones_ln, sqt[:],
                                 start=(o == 0), stop=(o == NCH - 1))
            mu = p1.tile([1, M2], F32, tag="mu", name="mu")
            mu2 = p1.tile([1, M2], F32, tag="mu2", name="mu2")
            var = p1.tile([1, M2], F32, tag="var", name="var")
            std = p1.tile([1, M2], F32, tag="std", name="std")
            istd = p1.tile([1, M2], F32, tag="istd", name="istd")
            nc.scalar.mul(mu[:], sum_ps[:], 1.0 / C)
            nc.vector.tensor_mul(mu2[:], mu[:], mu[:])
            nc.vector.scalar_tensor_tensor(var[:], ssq_ps[:], 1.0 / C, mu2[:],
                                           ALU.mult, ALU.subtract)
            eps_t = p1.tile([1, 1], F32, tag="eps", name="eps")
            nc.vector.memset(eps_t[:], 1e-5 * WS * WS)
            nc.scalar.activation(std[:], var[:], AF.Sqrt, bias=eps_t[:])
            nc.vector.reciprocal(istd[:], std[:])
            mu_b = p1.tile([128, M2], F32, tag="mu_b", name="mu_b")
            istd_b = p1.tile([128, M2], F32, tag="istd_b", name="istd_b")
            nc.gpsimd.partition_broadcast(mu_b[:], mu[:])
            nc.gpsimd.partition_broadcast(istd_b[:], istd[:])

            for o in range(NCH):
                t = p1q.tile([128, M2], F32, tag="lnt", name="lnt")
                nc.vector.tensor_sub(t[:], xr_sb[o][:], mu_b[:])
                nc.vector.tensor_mul(t[:], t[:], istd_b[:])
                nc.vector.tensor_scalar(xln2[:, o // 2, o % 2, :],
                                        t[:], lng_c(o), lnb_c(o),
                                        ALU.mult, ALU.add)

            # ---- K projection (fp8 DR) + RoPE ----
            k_sb = [p1.tile([128, M2], BF16, tag=f"ks{o}", name=f"ks{o}")
                    for o in range(NCH)]
            for o in range(NCH):
                k_ps = ps1.tile([128, M2], F32, tag=f"xr{o}", name=f"k{o}")
                mm_chain(k_ps[:],
                         lambda j, t: dr_slice(wk_t, j, t,
                                               slice(o * 128, (o + 1) * 128)),
                         lambda j, t: dr_slice(xln2, j, t, slice(0, M2)),
                         3, "kv" in FP8_PATHS)
                nc.scalar.activation(k_sb[o][:], k_ps[:], AF.Identity)
            for o in range(3):
                t1 = p1q.tile([128, M2], BF16, tag="kt1", name="kt1")
                t2 = p1q.tile([128, M2], BF16, tag="kt2", name="kt2")
                nc.gpsimd.tensor_mul(t1[:], k_sb[o][:], ck_t[:])
                nc.gpsimd.tensor_mul(t2[:], k_sb[o + 3][:], sk_t[:])
                nc.gpsimd.tensor_sub(rotk[o][:], t1[:], t2[:])
                t3 = p1q.tile([128, M2], BF16, tag="kt3", name="kt3")
                t4 = p1q.tile([128, M2], BF16, tag="kt4", name="kt4")
                nc.gpsimd.tensor_mul(t3[:], k_sb[o + 3][:], ck_t[:])
                nc.gpsimd.tensor_mul(t4[:], k_sb[o][:], sk_t[:])
                nc.gpsimd.tensor_add(rotk[o + 3][:], t3[:], t4[:])

            # ---- V projection (fp8 DR) ----
            for b in range(B):
                ms = slice(b * M, (b + 1) * M)
                for half in range(2):
                    v_ps = ps1.tile([M, 384], F32, tag=f"qp{half}",
                                    name=f"v{b}{half}")
                    mm_chain(v_ps[:],
                             lambda j, t: dr_slice(xln2, j, t, ms),
                             lambda j, t: dr_slice(
                                 wv_t, j, t,
                                 slice(half * 384, (half + 1) * 384)),
                             3, "kv" in FP8_PATHS)
                    nc.scalar.activation(vaug[b][:, half * 384:(half + 1) * 384],
                                         v_ps[:], AF.Identity)

        # ========== Phase 3: attention (batch 0 fully first so the
        # batch-0 output projection overlaps batch-1 attention) ==========
        with tc.tile_pool(name="p3", bufs=1) as p3, \
             tc.tile_pool(name="ps3", bufs=1, space="PSUM") as ps3, \
             tc.tile_pool(name="p4", bufs=2) as p4, \
             tc.tile_pool(name="ps4", bufs=1, space="PSUM") as ps4:

            def oproj_half(half):
                ns = slice(half * NQ, (half + 1) * NQ)
                for oc in range(NCH):
                    y_sb = p4.tile([128, NQ], BF16, tag="y", name=f"y{oc}{half}")
                    y_ps = ps4.tile([128, NQ], F32, tag="yp0",
                                    name=f"yp{oc}{half}")
                    mm_chain(y_ps[:],
                             lambda j, t: dr_slice(
                                 pj_t, j, t, slice(oc * 128, (oc + 1) * 128)),
                             lambda j, t: dr_slice(att2, j, t, ns),
                             3, "att" in FP8_PATHS)
                    nc.vector.tensor_scalar(y_sb[:], y_ps[:],
                                            1.0 / (WS * WS), pb_c(oc),
                                            ALU.mult, ALU.add)
                    nc.sync.dma_start(out=Y[oc * 128:(oc + 1) * 128, ns],
                                      in_=y_sb[:])

            pairs = [(h, b) for b in range(B) for h in range(HEADS)]
            e_sb = {}
            u_sb = {}
            for g in range(6):          # groups of 4 (h,b) pairs
                den_ps = ps3.tile([128, NQ], F32, tag=f"dn{g % 2}",
                                  name=f"dn{g}")
                for j in range(4):
                    h, b = pairs[4 * g + j]
                    hq, rs = h // 4, slice((h % 4) * 32, (h % 4) * 32 + 32)
                    ms = slice(b * M, (b + 1) * M)
                    qs = slice(b * NQ, (b + 1) * NQ)
                    z_ps = ps3.tile([M, NQ], F32, tag=f"z{j % 2}",
                                    name=f"z{g}{j}")
                    tp = ((h % 4) * 32, 0) if h % 4 == 3 else None
                    nc.tensor.matmul(z_ps[:], rotk[hq][rs, ms],
                                     rotq[hq][rs, qs], start=True, stop=False,
                                     tile_position=tp)
                    nc.tensor.matmul(z_ps[:], rotk[hq + 3][rs, ms],
                                     rotq[hq + 3][rs, qs],
                                     start=False, stop=False,
                                     tile_position=tp)
                    nc.tensor.matmul(z_ps[:], tri_t[:], oh_t[:],
                                     start=False, stop=True, perf_mode=DR,
                                     skip_group_check=True)
                    e = p3.tile([M, NQ], BF16, tag=f"e{j}", name=f"e{g}{j}")
                    nc.scalar.activation(e[:], z_ps[:], AF.Exp, scale=SCALE)
                    e_sb[(h, b)] = e
                    nc.tensor.matmul(den_ps[:],
                                     denoh_t[:, j * 128:(j + 1) * 128],
                                     e[:], start=(j == 0), stop=(j == 3))
                    u_ps = ps3.tile([HD, NQ], F32, tag=f"u{j % 2}",
                                    name=f"u{g}{j}")
                    u_sb[j] = u_ps
                    nc.tensor.matmul(u_ps[:], vaug[b][:, h * HD:(h + 1) * HD],
                                     e[:], start=True, stop=True)
                rec = p3.tile([128, NQ], BF16, tag=f"rc{g % 2}",
                              name=f"rc{g}")
                with nc.allow_low_precision(reason="softmax recip bf16"):
                    nc.vector.reciprocal(rec[:], den_ps[:])
                # rebuild 1/den as [128, 512] (two 64-row pair blocks) and
                # evacuate to SBUF so the division has a single PSUM input
                rb_sb = []
                for half2 in range(2):
                    bc_ps = ps3.tile([128, NQ], F32, tag="bc0",
                                     name=f"bc{g}{half2}")
                    nc.tensor.matmul(bc_ps[:],
                                     sel2_t[64 * half2:64 * half2 + 2, :],
                                     rec[64 * half2:64 * half2 + 2, :],
                                     start=True, stop=True)
                    rb = p3.tile([128, NQ], BF16, tag=f"rb{half2}",
                                 name=f"rb{g}{half2}")
                    if half2 == 0:
                        nc.scalar.activation(rb[:], bc_ps[:], AF.Identity)
                    else:
                        nc.vector.tensor_copy(rb[:], bc_ps[:])
                    rb_sb.append(rb)
                for j in range(4):
                    h, b = pairs[4 * g + j]
                    qs = slice(b * NQ, (b + 1) * NQ)
                    e_sb.pop((h, b))
                    u_ps = u_sb.pop(j)
                    rbrow = rb_sb[j // 2][(j % 2) * 64:(j % 2) * 64 + 64, :]
                    dst = att2[(h % 2) * 64:(h % 2) * 64 + 64,
                               h // 4, (h // 2) % 2, qs]
                    nc.vector.tensor_mul(dst, u_ps[:], rbrow)
                if g == 2:
                    oproj_half(0)
            oproj_half(1)

    nc.compile()
    return nc


# ======================= host-side preparation =======================

def _angles(dim, end, w, step=1.0, bias=0.0, theta=10000.0):
    flat = np.arange(end, dtype=np.float32)
    xp = (bias + (flat % w) * step).astype(np.float32)
    yp = (bias + (flat // w) * step).astype(np.float32)
    freqs = (1.0 / theta ** (np.arange(0, dim, 4, dtype=np.float32)[: dim // 4]
                             / dim)).astype(np.float32)
    xf = np.outer(xp, freqs)
    yf = np.outer(yp, freqs)
    return np.stack([xf, yf], axis=-1).reshape(end, -1).astype(np.float32)


def _dr_pack(mat, nblk, fp8):
    """[K, F] f32 -> [128, nblk, 2, F] with k = (2j+t)*128+p."""
    K, F = mat.shape
    assert K == nblk * 256
    out = mat.reshape(nblk, 2, 128, F).transpose(2, 0, 1, 3)
    if fp8:
        return np.ascontiguousarray(np.clip(out, -240, 240)).astype(NP_FP8)
    return np.ascontiguousarray(out).astype(NP_BF16)


def _host_prep(x, Wq, Wkv, sr_w, sr_b, ln_g, ln_b, proj_w, proj_b):
    f = np.float32
    x = np.asarray(x, f)
    Wq = np.asarray(Wq, f)
    Wkv = np.asarray(Wkv, f)
    sr_w = np.asarray(sr_w, f)
    proj_w = np.asarray(proj_w, f)

    # pair-split permutation: rows 0..383 pair-first, 384..767 pair-second
    hh = np.arange(HEADS)[:, None] * HD
    jj = np.arange(HD // 2)[None, :] * 2
    perm = np.concatenate([(hh + jj).ravel(), (hh + jj + 1).ravel()])

    f8q = "q" in FP8_PATHS
    f8kv = "kv" in FP8_PATHS
    f8cv = "conv" in FP8_PATHS
    f8at = "att" in FP8_PATHS
    wq_dr = _dr_pack(WS * Wq[perm, :].T, 3, f8q)         # [c_in, out-perm]
    wk_dr = _dr_pack(WS * Wkv[:C][perm, :].T, 3, f8kv)
    wv_dr = _dr_pack(WS * Wkv[C:].T, 3, f8kv)
    srw_dr = _dr_pack(WS * sr_w.reshape(C, KC).T, 12, f8cv)  # [kc, out]
    pj_dr = _dr_pack(WS * proj_w.T, 3, f8at)             # [c_att, out]

    # im2col of the first 6 image rows, both batches: [3072, 192]
    strip = x[:, :6 * W, :].reshape(B, 3, 2, 32, 2, C)   # b, i, di, j, dj, c
    xi2c = strip.transpose(5, 2, 4, 0, 1, 3).reshape(KC, M2)
    xi_dr = _dr_pack(xi2c, 12, f8cv)

    # RoPE tables (per-row freq pattern), q tables folded with 1/WS
    ang_q = _angles(HD, N, W)
    ang_k = _angles(HD, N // (SR * SR), W, step=SR, bias=1.0 - 1.0 / SR)
    rowj = np.arange(128) % 32
    cq_full = (np.cos(ang_q)[:, rowj].T / WS).astype(NP_BF16)   # [128, 4096]
    sq_full = (np.sin(ang_q)[:, rowj].T / WS).astype(NP_BF16)
    ckk = np.cos(ang_k)[:M, rowj].T / WS
    skk = np.sin(ang_k)[:M, rowj].T / WS
    ck2 = np.ascontiguousarray(np.concatenate([ckk, ckk], 1)).astype(NP_BF16)
    sk2 = np.ascontiguousarray(np.concatenate([skk, skk], 1)).astype(NP_BF16)

    # visibility
    n_all = np.arange(N)
    xpos = n_all // (SR * H)
    ox = n_all // H
    oy = n_all % H
    vis = xpos * SR + (ox + oy * H) // (SR * H) + 1       # [4096], <= 94

    # mask matmul stationary: tri[kk, m] = NEG if m >= kk+1
    tri = np.where(np.arange(M)[None, :] >= np.arange(M)[:, None] + 1,
                   NEG, 0.0).astype(f)                    # [96, 96]
    tri_drp = np.ascontiguousarray(
        tri.reshape(2, 48, M).transpose(1, 0, 2)).astype(NP_FP8)

    # den one-hot stationary: block j puts den_j at psum partition
    # {0,1,64,65}[j]; block 0 all ones except those (keeps rows finite)
    dencol = [0, 1, 64, 65]
    denoh = np.zeros((M, 512), f)
    denoh[:, 0:128] = 1.0
    denoh[:, dencol[1:]] = 0.0
    for j in range(1, 4):
        denoh[:, 128 * j + dencol[j]] = 1.0
    denoh = denoh.astype(NP_BF16)

    sel2v = np.zeros((128, 128), f)
    for base in (0, 64):
        sel2v[base + 0, 0:64] = 1.0
        sel2v[base + 1, 64:128] = 1.0
    sel2v = sel2v.astype(NP_BF16)

    onesb = np.ones((128, 65), NP_BF16)  # col0: LN ones

    biases = np.zeros((128, 4 * NCH), f)
    biases[:, 0:NCH] = (WS * np.asarray(sr_b, f)).reshape(NCH, 128).T
    biases[:, NCH:2 * NCH] = np.asarray(ln_g, f).reshape(NCH, 128).T
    biases[:, 2 * NCH:3 * NCH] = np.asarray(ln_b, f).reshape(NCH, 128).T
    biases[:, 3 * NCH:4 * NCH] = np.asarray(proj_b, f).reshape(NCH, 128).T

    shared = dict(wq_dr=wq_dr, wk_dr=wk_dr, wv_dr=wv_dr, srw_dr=srw_dr,
                  xi_dr=xi_dr, pj_dr=pj_dr, ck=ck2, sk=sk2, tri_dr=tri_drp,
                  denoh=denoh, sel2=sel2v, onesb=onesb, biases=biases)

    in_maps = []
    for core in range(N_CORES):
        ns = slice(core * NQ, (core + 1) * NQ)
        xs = x[:, ns, :]                                  # [2, 512, 768]
        xT = xs.transpose(2, 0, 1).reshape(C, NQT)
        xT_drp = _dr_pack(xT, 3, f8q)
        cqc = np.ascontiguousarray(
            np.concatenate([cq_full[:, ns]] * B, axis=1))
        sqc = np.ascontiguousarray(
            np.concatenate([sq_full[:, ns]] * B, axis=1))
        oh = (np.arange(M)[:, None] == (vis[ns] - 1)[None, :]).astype(f)
        oh_drp = np.ascontiguousarray(
            oh.reshape(2, 48, NQ).transpose(1, 0, 2)).astype(NP_FP8)
        in_maps.append(dict(shared, xT_dr=xT_drp, cq=cqc, sq=sqc,
                            oh_dr=oh_drp))
    return in_maps


_NC_CACHE = {}


def _get_program():
    if "nc" not in _NC_CACHE:
        _NC_CACHE["nc"] = build_program()
    return _NC_CACHE["nc"]


def kernel(x, Wq, Wkv, sr_w, sr_b, ln_g, ln_b, proj_w, proj_b, H=None, W=None,
           _trace=False):
    nc = _get_program()
    in_maps = _host_prep(x, Wq, Wkv, sr_w, sr_b, ln_g, ln_b, proj_w, proj_b)
    res = run_bass_kernel_spmd(nc, in_maps, list(range(N_CORES)),
                               trace=_trace)
    kernel.last_result = res
    out = np.empty((B, N, C), np.float32)
    for core in range(N_CORES):
        yT = np.asarray(res.results[core]["y"]).astype(np.float32)  # [768,1024]
        y = yT.reshape(C, B, NQ).transpose(1, 2, 0)       # [B, 512, 768]
        out[:, core * NQ:(core + 1) * NQ, :] = y
    return out


# revision 22
# speedup vs baseline: 1.0802x; 1.0802x over previous
"""Trainium2 Bass kernel for nn_GSAttention (spatial-reduction attention).

Strategy (v2)
-------------
* Queries sharded 512/core x 8 cores; B=2 kept on-core (1024 query rows).
* Only the first 96 reduced KV tokens are visible (max vis = 94), so the
  whole KV path (conv + LN + KV proj) runs on 96 tokens, replicated.
* All big matmuls in fp8e4m3 DoubleRow mode (4x bf16 throughput):
  q-proj, conv, k-proj, v-proj, out-proj, and the additive causal mask
  (one-hot matmul, values 0/-240 exact in fp8).  QK / AV / softmax-denom /
  denom-broadcast matmuls in bf16.  Weights are host-scaled x64 to clear
  the fp8 subnormal range; the scale is folded back into the RoPE tables
  (q/k paths) and the final output-evacuation scale (v/proj paths).
* Mask as matmul: z += tri^T @ onehot(vis) accumulated into the QK PSUM
  group; exp then masks to ~1e-13 naturally.
* Softmax denominators: per (head,batch) pair j in a group of 4, a
  [96,128] one-hot-column stationary places den_j at PSUM partition 32j;
  one DVE reciprocal per group; per-pair broadcast matmul (ones row at
  partition 32j x rec row) rebuilds [64,512] 1/den; the division fuses
  the PSUM->SBUF evacuation of AV into the DR-packed fp8 att2 tile.
* Out-proj in [c_out, q] orientation so the projection bias and the
  1/4096 fp8-scale fold into the Activation-engine PSUM evacuation.
  Host transposes the returned [768, 1024] tiles.
* One DMA per tensor, host pre-packed in device layouts (fp8/bf16):
  ~8.5 MB total HBM traffic per core.
"""

import os
import sys

for _p in ("/opt/trn_rl_repo", "/root/.axon_site/_ro/trn_rl_repo"):
    if os.path.isdir(_p) and _p not in sys.path:
        sys.path.insert(0, _p)

from contextlib import ExitStack

import numpy as np
import ml_dtypes

import types as _types
if "antenv.axon_hooks" not in sys.modules:
    _axh = _types.ModuleType("antenv.axon_hooks")
    _axh.get_axon_ntff_profile_hook = lambda: None
    sys.modules["antenv.axon_hooks"] = _axh

import concourse.bacc as bacc
import concourse.mybir as mybir
from concourse.tile import TileContext
from concourse.bass_utils import run_bass_kernel_spmd

F32 = mybir.dt.float32
BF16 = mybir.dt.bfloat16
FP8 = mybir.dt.float8e4
AF = mybir.ActivationFunctionType
ALU = mybir.AluOpType
DR = mybir.MatmulPerfMode.DoubleRow

NP_BF16 = ml_dtypes.bfloat16
NP_FP8 = ml_dtypes.float8_e4m3

# Problem constants.
N_CORES = 8
B = 2
N = 4096
C = 768
HEADS = 12
HD = 64
SR = 2
H = W = 64
NQ = 512            # queries per core per batch
NQT = B * NQ        # query rows per core
M = 96              # padded visible reduced tokens (real max vis = 94)
M2 = B * M
KC = C * SR * SR    # 3072 conv contraction
SCALE = 1.0 / 8.0
NEG = -240.0        # fp8e4m3 max; exp(scale*-240) ~ 1e-13
NCH = C // 128      # 6 feature chunks

# fp8 quantization of any data path costs 2.6-4% max-rel-err (threshold 2e-2),
# so data matmuls run bf16; only the exact-valued mask matmul uses fp8 DR.
FP8_PATHS = set()
WS = 64.0 if FP8_PATHS else 1.0


def build_program():
    nc = bacc.Bacc("TRN2", target_bir_lowering=False, debug=False,
                   num_devices=N_CORES)

    def par(name, shape, dt, out=False):
        return nc.declare_dram_parameter(name, list(shape), dt, isOutput=out)

    # host-packed inputs (one DMA each)
    DT_Q = FP8 if "q" in FP8_PATHS else BF16
    DT_CONV = FP8 if "conv" in FP8_PATHS else BF16
    DT_KV = FP8 if "kv" in FP8_PATHS else BF16
    DT_ATT = FP8 if "att" in FP8_PATHS else BF16
    xT_dr = par("xT_dr", (128, 3, 2, NQT), DT_Q)
    wq_dr = par("wq_dr", (128, 3, 2, C), DT_Q)
    wk_dr = par("wk_dr", (128, 3, 2, C), DT_KV)
    wv_dr = par("wv_dr", (128, 3, 2, C), DT_KV)
    srw_dr = par("srw_dr", (128, 12, 2, C), DT_CONV)
    xi_dr = par("xi_dr", (128, 12, 2, M2), DT_CONV)
    pj_dr = par("pj_dr", (128, 3, 2, C), DT_ATT)
    cq = par("cq", (128, NQT), BF16)
    sq = par("sq", (128, NQT), BF16)
    ck = par("ck", (128, M2), BF16)
    sk = par("sk", (128, M2), BF16)
    tri_dr = par("tri_dr", (48, 2, M), FP8)
    oh_dr = par("oh_dr", (48, 2, NQ), FP8)
    denoh = par("denoh", (M, 512), BF16)
    sel2 = par("sel2", (128, 128), BF16)
    onesb = par("onesb", (128, 65), BF16)
    biases = par("biases", (128, 4 * NCH), F32)   # srb | lng | lnb | pbias
    Y = par("y", (C, NQT), BF16, out=True)

    with TileContext(nc) as tc, ExitStack() as st:
        pers = st.enter_context(tc.tile_pool(name="pers", bufs=1))

        # ---- persistent tiles ----
        cq_t = pers.tile([128, NQT], BF16, tag="cq", name="cq")
        sq_t = pers.tile([128, NQT], BF16, tag="sq", name="sq")
        ck_t = pers.tile([128, M2], BF16, tag="ck", name="ck")
        sk_t = pers.tile([128, M2], BF16, tag="sk", name="sk")
        tri_t = pers.tile([48, 2, M], FP8, tag="tri", name="tri")
        oh_t = pers.tile([48, 2, NQ], FP8, tag="oh", name="oh")
        denoh_t = pers.tile([M, 512], BF16, tag="denoh", name="denoh")
        sel2_t = pers.tile([128, 128], BF16, tag="sel2", name="sel2")
        onesb_t = pers.tile([128, 65], BF16, tag="onesb", name="onesb")
        bias_t = pers.tile([128, 4 * NCH], F32, tag="bias", name="bias")
        xT_t = pers.tile([128, 3, 2, NQT], DT_Q, tag="xT", name="xT")
        wq_t = pers.tile([128, 3, 2, C], DT_Q, tag="wq", name="wq")
        wk_t = pers.tile([128, 3, 2, C], DT_KV, tag="wk", name="wk")
        wv_t = pers.tile([128, 3, 2, C], DT_KV, tag="wv", name="wv")
        srw_t = pers.tile([128, 12, 2, C], DT_CONV, tag="srw", name="srw")
        xi_t = pers.tile([128, 12, 2, M2], DT_CONV, tag="xi", name="xi")
        pj_t = pers.tile([128, 3, 2, C], DT_ATT, tag="pj", name="pj")
        rotq = [pers.tile([128, NQT], BF16, tag=f"rotq{i}", name=f"rotq{i}")
                for i in range(NCH)]
        rotk = [pers.tile([128, M2], BF16, tag=f"rotk{i}", name=f"rotk{i}")
                for i in range(NCH)]
        xln2 = pers.tile([128, 3, 2, M2], DT_KV, tag="xln2", name="xln2")
        vaug = [pers.tile([M, C], BF16, tag=f"vaug{b}", name=f"vaug{b}")
                for b in range(B)]
        att2 = pers.tile([128, 3, 2, NQT], DT_ATT, tag="att2", name="att2")

        # DMA priority order: q path first, then KV path, proj last.
        nc.sync.dma_start(out=bias_t[:], in_=biases[:])
        for j in range(3):
            nc.sync.dma_start(out=xT_t[:, j, :, :], in_=xT_dr[:, j, :, :])
            nc.sync.dma_start(out=wq_t[:, j, :, :], in_=wq_dr[:, j, :, :])
        nc.sync.dma_start(out=cq_t[:], in_=cq[:])
        nc.sync.dma_start(out=sq_t[:], in_=sq[:])
        nc.sync.dma_start(out=xi_t[:], in_=xi_dr[:])
        for g in range(2):
            nc.sync.dma_start(out=srw_t[:, 6 * g:6 * (g + 1), :, :],
                              in_=srw_dr[:, 6 * g:6 * (g + 1), :, :])
        nc.sync.dma_start(out=wk_t[:], in_=wk_dr[:])
        nc.sync.dma_start(out=wv_t[:], in_=wv_dr[:])
        nc.sync.dma_start(out=ck_t[:], in_=ck[:])
        nc.sync.dma_start(out=sk_t[:], in_=sk[:])
        nc.sync.dma_start(out=tri_t[:], in_=tri_dr[:])
        nc.sync.dma_start(out=oh_t[:], in_=oh_dr[:])
        nc.sync.dma_start(out=denoh_t[:], in_=denoh[:])
        nc.sync.dma_start(out=sel2_t[:], in_=sel2[:])
        nc.sync.dma_start(out=onesb_t[:], in_=onesb[:])
        nc.sync.dma_start(out=pj_t[:], in_=pj_dr[:])

        def mm_chain(out_ap, stat_fn, mov_fn, nblk, fp8, tp=None):
            """Accumulate out += stat_j^T @ mov_j over nblk k-blocks."""
            for j in range(nblk):
                if fp8:
                    nc.tensor.matmul(out_ap, stat_fn(j, None), mov_fn(j, None),
                                     start=(j == 0), stop=(j == nblk - 1),
                                     perf_mode=DR, tile_position=tp)
                else:
                    for t in range(2):
                        nc.tensor.matmul(out_ap, stat_fn(j, t), mov_fn(j, t),
                                         start=(j == 0 and t == 0),
                                         stop=(j == nblk - 1 and t == 1),
                                         tile_position=tp)

        def dr_slice(tile, j, t, cols):
            return tile[:, j, :, cols] if t is None else tile[:, j, t, cols]

        srb_c = lambda o: bias_t[:, o:o + 1]
        lng_c = lambda o: bias_t[:, NCH + o:NCH + o + 1]
        lnb_c = lambda o: bias_t[:, 2 * NCH + o:2 * NCH + o + 1]
        pb_c = lambda o: bias_t[:, 3 * NCH + o:3 * NCH + o + 1]

        # ========== Phase 1: q-projection + conv (fp8 DR) ==========
        with tc.tile_pool(name="p1", bufs=1) as p1, \
             tc.tile_pool(name="p1q", bufs=2) as p1q, \
             tc.tile_pool(name="ps1", bufs=1, space="PSUM") as ps1:
            q_sb = [p1.tile([128, NQT], BF16, tag=f"qsb{i}", name=f"qsb{i}")
                    for i in range(NCH)]

            def q_block(cc, half):
                ns = slice(half * NQ, (half + 1) * NQ)
                q_ps = ps1.tile([128, NQ], F32, tag=f"qp{(2 * cc + half) % 2}",
                                name=f"qp{cc}{half}")
                mm_chain(q_ps[:],
                         lambda j, t: dr_slice(wq_t, j, t,
                                               slice(cc * 128, (cc + 1) * 128)),
                         lambda j, t: dr_slice(xT_t, j, t, ns),
                         3, "q" in FP8_PATHS)
                nc.scalar.activation(q_sb[cc][:, ns], q_ps[:], AF.Identity)

            xr_ps = [ps1.tile([128, M2], F32, tag=f"xr{o}", name=f"xr{o}")
                     for o in range(NCH)]

            conv_fp8 = "conv" in FP8_PATHS

            def conv_col(o, jlo, jhi):
                for j2 in range(jlo, jhi):
                    if conv_fp8:
                        nc.tensor.matmul(
                            xr_ps[o][:], srw_t[:, j2, :, o * 128:(o + 1) * 128],
                            xi_t[:, j2, :, :], start=(j2 == 0), stop=(j2 == 11),
                            perf_mode=DR)
                    else:
                        for t in range(2):
                            nc.tensor.matmul(
                                xr_ps[o][:],
                                srw_t[:, j2, t, o * 128:(o + 1) * 128],
                                xi_t[:, j2, t, :],
                                start=(j2 == 0 and t == 0),
                                stop=(j2 == 11 and t == 1))

            # interleave: 12 q blocks with 12 conv half-columns
            for step in range(12):
                q_block(step // 2, step % 2)
                o, hcol = step % 6, step // 6
                conv_col(o, 6 * hcol, 6 * (hcol + 1))

            # ---- RoPE on q (bf16 2x on DVE; half-ops split with Pool) ----
            for o in range(3):
                t1 = p1q.tile([128, NQT], BF16, tag="rt1", name="rt1")
                t2 = p1q.tile([128, NQT], BF16, tag="rt2", name="rt2")
                nc.vector.tensor_mul(t1[:], q_sb[o][:], cq_t[:])
                nc.vector.tensor_mul(t2[:], q_sb[o + 3][:], sq_t[:])
                nc.vector.tensor_sub(rotq[o][:], t1[:], t2[:])
                t3 = p1q.tile([128, NQT], BF16, tag="rt3", name="rt3")
                t4 = p1q.tile([128, NQT], BF16, tag="rt4", name="rt4")
                nc.vector.tensor_mul(t3[:], q_sb[o + 3][:], cq_t[:])
                nc.vector.tensor_mul(t4[:], q_sb[o][:], sq_t[:])
                nc.vector.tensor_add(rotq[o + 3][:], t3[:], t4[:])

            # ========== Phase 2: LN + K/V ==========
            xr_sb = [p1.tile([128, M2], BF16, tag=f"xs{o}", name=f"xs{o}")
                     for o in range(NCH)]
            for o in range(NCH):
                nc.scalar.activation(xr_sb[o][:], xr_ps[o][:],
                                     AF.Identity, bias=srb_c(o))

            ones_ln = onesb_t[:, 0:1]
            sum_ps = ps1.tile([1, M2], F32, tag="xr0", name="sum")
            for o in range(NCH):
                nc.tensor.matmul(sum_ps[:], ones_ln, xr_sb[o][:],
                                 start=(o == 0), stop=(o == NCH - 1))
            ssq_ps = ps1.tile([1, M2], F32, tag="xr1", name="ssq")
            for o in range(NCH):
                sqt = p1q.tile([128, M2], BF16, tag="sqt", name="sqt")
                nc.gpsimd.tensor_mul(sqt[:], xr_sb[o][:], xr_sb[o][:])
                nc.tensor.matmul(ssq_ps[:], ones_ln, sqt[:],
                                 start=(o == 0), stop=(o == NCH - 1))
            mu = p1.tile([1, M2], F32, tag="mu", name="mu")
            mu2 = p1.tile([1, M2], F32, tag="mu2", name="mu2")
            var = p1.tile([1, M2], F32, tag="var", name="var")
            std = p1.tile([1, M2], F32, tag="std", name="std")
            istd = p1.tile([1, M2], F32, tag="istd", name="istd")
            nc.scalar.mul(mu[:], sum_ps[:], 1.0 / C)
            nc.vector.tensor_mul(mu2[:], mu[:], mu[:])
            nc.vector.scalar_tensor_tensor(var[:], ssq_ps[:], 1.0 / C, mu2[:],
                                           ALU.mult, ALU.subtract)
            eps_t = p1.tile([1, 1], F32, tag="eps", name="eps")
            nc.vector.memset(eps_t[:], 1e-5 * WS * WS)
            nc.scalar.activation(std[:], var[:], AF.Sqrt, bias=eps_t[:])
            nc.vector.reciprocal(istd[:], std[:])
            mu_b = p1.tile([128, M2], F32, tag="mu_b", name="mu_b")
            istd_b = p1.tile([128, M2], F32, tag="istd_b", name="istd_b")
            nc.gpsimd.partition_broadcast(mu_b[:], mu[:])
            nc.gpsimd.partition_broadcast(istd_b[:], istd[:])

            for o in range(NCH):
                t = p1q.tile([128, M2], F32, tag="lnt", name="lnt")
                nc.vector.tensor_sub(t[:], xr_sb[o][:], mu_b[:])
                nc.vector.tensor_mul(t[:], t[:], istd_b[:])
                nc.vector.tensor_scalar(xln2[:, o // 2, o % 2, :],
                                        t[:], lng_c(o), lnb_c(o),
                                        ALU.mult, ALU.add)

            # ---- K projection (fp8 DR) + RoPE ----
            k_sb = [p1.tile([128, M2], BF16, tag=f"ks{o}", name=f"ks{o}")
                    for o in range(NCH)]
            for o in range(NCH):
                k_ps = ps1.tile([128, M2], F32, tag=f"xr{o}", name=f"k{o}")
                mm_chain(k_ps[:],
                         lambda j, t: dr_slice(wk_t, j, t,
                                               slice(o * 128, (o + 1) * 128)),
                         lambda j, t: dr_slice(xln2, j, t, slice(0, M2)),
                         3, "kv" in FP8_PATHS)
                nc.scalar.activation(k_sb[o][:], k_ps[:], AF.Identity)
            for o in range(3):
                t1 = p1q.tile([128, M2], BF16, tag="kt1", name="kt1")
                t2 = p1q.tile([128, M2], BF16, tag="kt2", name="kt2")
                nc.gpsimd.tensor_mul(t1[:], k_sb[o][:], ck_t[:])
                nc.gpsimd.tensor_mul(t2[:], k_sb[o + 3][:], sk_t[:])
                nc.gpsimd.tensor_sub(rotk[o][:], t1[:], t2[:])
                t3 = p1q.tile([128, M2], BF16, tag="kt3", name="kt3")
                t4 = p1q.tile([128, M2], BF16, tag="kt4", name="kt4")
                nc.gpsimd.tensor_mul(t3[:], k_sb[o + 3][:], ck_t[:])
                nc.gpsimd.tensor_mul(t4[:], k_sb[o][:], sk_t[:])
                nc.gpsimd.tensor_add(rotk[o + 3][:], t3[:], t4[:])

            # ---- V projection (fp8 DR) ----
            for b in range(B):
                ms = slice(b * M, (b + 1) * M)
                for half in range(2):
                    v_ps = ps1.tile([M, 384], F32, tag=f"qp{half}",
                                    name=f"v{b}{half}")
                    mm_chain(v_ps[:],
                             lambda j, t: dr_slice(xln2, j, t, ms),
                             lambda j, t: dr_slice(
                                 wv_t, j, t,
                                 slice(half * 384, (half + 1) * 384)),
                             3, "kv" in FP8_PATHS)
                    nc.scalar.activation(vaug[b][:, half * 384:(half + 1) * 384],
                                         v_ps[:], AF.Identity)

        # ========== Phase 3: attention (batch 0 fully first so the
        # batch-0 output projection overlaps batch-1 attention) ==========
        with tc.tile_pool(name="p3", bufs=1) as p3, \
             tc.tile_pool(name="ps3", bufs=1, space="PSUM") as ps3, \
             tc.tile_pool(name="p4", bufs=2) as p4, \
             tc.tile_pool(name="ps4", bufs=1, space="PSUM") as ps4:

            def oproj_half(half):
                ns = slice(half * NQ, (half + 1) * NQ)
                for oc in range(NCH):
                    y_sb = p4.tile([128, NQ], BF16, tag="y", name=f"y{oc}{half}")
                    y_ps = ps4.tile([128, NQ], F32, tag="yp0",
                                    name=f"yp{oc}{half}")
                    mm_chain(y_ps[:],
                             lambda j, t: dr_slice(
                                 pj_t, j, t, slice(oc * 128, (oc + 1) * 128)),
                             lambda j, t: dr_slice(att2, j, t, ns),
                             3, "att" in FP8_PATHS)
                    nc.vector.tensor_scalar(y_sb[:], y_ps[:],
                                            1.0 / (WS * WS), pb_c(oc),
                                            ALU.mult, ALU.add)
                    nc.sync.dma_start(out=Y[oc * 128:(oc + 1) * 128, ns],
                                      in_=y_sb[:])

            pairs = [(h, b) for b in range(B) for h in range(HEADS)]
            e_sb = {}
            u_sb = {}
            for g in range(6):          # groups of 4 (h,b) pairs
                den_ps = ps3.tile([128, NQ], F32, tag=f"dn{g % 2}",
                                  name=f"dn{g}")
                for j in range(4):
                    h, b = pairs[4 * g + j]
                    hq, rs = h // 4, slice((h % 4) * 32, (h % 4) * 32 + 32)
                    ms = slice(b * M, (b + 1) * M)
                    qs = slice(b * NQ, (b + 1) * NQ)
                    z_ps = ps3.tile([M, NQ], F32, tag=f"z{j % 2}",
                                    name=f"z{g}{j}")
                    tp = ((h % 4) * 32, 0) if h % 4 == 3 else None
                    nc.tensor.matmul(z_ps[:], rotk[hq][rs, ms],
                                     rotq[hq][rs, qs], start=True, stop=False,
                                     tile_position=tp)
                    nc.tensor.matmul(z_ps[:], rotk[hq + 3][rs, ms],
                                     rotq[hq + 3][rs, qs],
                                     start=False, stop=False,
                                     tile_position=tp)
                    nc.tensor.matmul(z_ps[:], tri_t[:], oh_t[:],
                                     start=False, stop=True, perf_mode=DR,
                                     skip_group_check=True)
                    e = p3.tile([M, NQ], BF16, tag=f"e{j}", name=f"e{g}{j}")
                    nc.scalar.activation(e[:], z_ps[:], AF.Exp, scale=SCALE)
                    e_sb[(h, b)] = e
                    nc.tensor.matmul(den_ps[:],
                                     denoh_t[:, j * 128:(j + 1) * 128],
                                     e[:], start=(j == 0), stop=(j == 3))
                    u_ps = ps3.tile([HD, NQ], F32, tag=f"u{j % 2}",
                                    name=f"u{g}{j}")
                    u_sb[j] = u_ps
                    nc.tensor.matmul(u_ps[:], vaug[b][:, h * HD:(h + 1) * HD],
                                     e[:], start=True, stop=True)
                rec = p3.tile([128, NQ], BF16, tag=f"rc{g % 2}",
                              name=f"rc{g}")
                with nc.allow_low_precision(reason="softmax recip bf16"):
                    nc.vector.reciprocal(rec[:], den_ps[:])
                # rebuild 1/den as [128, 512] (two 64-row pair blocks) and
                # evacuate to SBUF so the division has a single PSUM input
                rb_sb = []
                for half2 in range(2):
                    bc_ps = ps3.tile([128, NQ], F32, tag="bc0",
                                     name=f"bc{g}{half2}")
                    nc.tensor.matmul(bc_ps[:],
                                     sel2_t[64 * half2:64 * half2 + 2, :],
                                     rec[64 * half2:64 * half2 + 2, :],
                                     start=True, stop=True)
                    rb = p3.tile([128, NQ], BF16, tag=f"rb{half2}",
                                 name=f"rb{g}{half2}")
                    if half2 == 0:
                        nc.scalar.activation(rb[:], bc_ps[:], AF.Identity)
                    else:
                        nc.vector.tensor_copy(rb[:], bc_ps[:])
                    rb_sb.append(rb)
                for j in range(4):
                    h, b = pairs[4 * g + j]
                    qs = slice(b * NQ, (b + 1) * NQ)
                    e_sb.pop((h, b))
                    u_ps = u_sb.pop(j)
                    rbrow = rb_sb[j // 2][(j % 2) * 64:(j % 2) * 64 + 64, :]
                    dst = att2[(h % 2) * 64:(h % 2) * 64 + 64,
                               h // 4, (h // 2) % 2, qs]
                    nc.vector.tensor_mul(dst, u_ps[:], rbrow)
                if g == 2:
                    oproj_half(0)
            oproj_half(1)

    nc.compile()
    return nc


# ======================= host-side preparation =======================

def _angles(dim, end, w, step=1.0, bias=0.0, theta=10000.0):
    flat = np.arange(end, dtype=np.float32)
    xp = (bias + (flat % w) * step).astype(np.float32)
    yp = (bias + (flat // w) * step).astype(np.float32)
    freqs = (1.0 / theta ** (np.arange(0, dim, 4, dtype=np.float32)[: dim // 4]
                             / dim)).astype(np.float32)
    xf = np.outer(xp, freqs)
    yf = np.outer(yp, freqs)
    return np.stack([xf, yf], axis=-1).reshape(end, -1).astype(np.float32)


def _dr_pack(mat, nblk, fp8):
    """[K, F] f32 -> [128, nblk, 2, F] with k = (2j+t)*128+p."""
    K, F = mat.shape
    assert K == nblk * 256
    out = mat.reshape(nblk, 2, 128, F).transpose(2, 0, 1, 3)
    if fp8:
        return np.ascontiguousarray(np.clip(out, -240, 240)).astype(NP_FP8)
    return np.ascontiguousarray(out).astype(NP_BF16)


def _host_prep(x, Wq, Wkv, sr_w, sr_b, ln_g, ln_b, proj_w, proj_b):
    f = np.float32
    x = np.asarray(x, f)
    Wq = np.asarray(Wq, f)
    Wkv = np.asarray(Wkv, f)
    sr_w = np.asarray(sr_w, f)
    proj_w = np.asarray(proj_w, f)

    # pair-split permutation: rows 0..383 pair-first, 384..767 pair-second
    hh = np.arange(HEADS)[:, None] * HD
    jj = np.arange(HD // 2)[None, :] * 2
    perm = np.concatenate([(hh + jj).ravel(), (hh + jj + 1).ravel()])

    f8q = "q" in FP8_PATHS
    f8kv = "kv" in FP8_PATHS
    f8cv = "conv" in FP8_PATHS
    f8at = "att" in FP8_PATHS
    wq_dr = _dr_pack(WS * Wq[perm, :].T, 3, f8q)         # [c_in, out-perm]
    wk_dr = _dr_pack(WS * Wkv[:C][perm, :].T, 3, f8kv)
    wv_dr = _dr_pack(WS * Wkv[C:].T, 3, f8kv)
    srw_dr = _dr_pack(WS * sr_w.reshape(C, KC).T, 12, f8cv)  # [kc, out]
    pj_dr = _dr_pack(WS * proj_w.T, 3, f8at)             # [c_att, out]

    # im2col of the first 6 image rows, both batches: [3072, 192]
    strip = x[:, :6 * W, :].reshape(B, 3, 2, 32, 2, C)   # b, i, di, j, dj, c
    xi2c = strip.transpose(5, 2, 4, 0, 1, 3).reshape(KC, M2)
    xi_dr = _dr_pack(xi2c, 12, f8cv)

    # RoPE tables (per-row freq pattern), q tables folded with 1/WS
    ang_q = _angles(HD, N, W)
    ang_k = _angles(HD, N // (SR * SR), W, step=SR, bias=1.0 - 1.0 / SR)
    rowj = np.arange(128) % 32
    cq_full = (np.cos(ang_q)[:, rowj].T / WS).astype(NP_BF16)   # [128, 4096]
    sq_full = (np.sin(ang_q)[:, rowj].T / WS).astype(NP_BF16)
    ckk = np.cos(ang_k)[:M, rowj].T / WS
    skk = np.sin(ang_k)[:M, rowj].T / WS
    ck2 = np.ascontiguousarray(np.concatenate([ckk, ckk], 1)).astype(NP_BF16)
    sk2 = np.ascontiguousarray(np.concatenate([skk, skk], 1)).astype(NP_BF16)

    # visibility
    n_all = np.arange(N)
    xpos = n_all // (SR * H)
    ox = n_all // H
    oy = n_all % H
    vis = xpos * SR + (ox + oy * H) // (SR * H) + 1       # [4096], <= 94

    # mask matmul stationary: tri[kk, m] = NEG if m >= kk+1
    tri = np.where(np.arange(M)[None, :] >= np.arange(M)[:, None] + 1,
                   NEG, 0.0).astype(f)                    # [96, 96]
    tri_drp = np.ascontiguousarray(
        tri.reshape(2, 48, M).transpose(1, 0, 2)).astype(NP_FP8)

    # den one-hot stationary: block j puts den_j at psum partition
    # {0,1,64,65}[j]; block 0 all ones except those (keeps rows finite)
    dencol = [0, 1, 64, 65]
    denoh = np.zeros((M, 512), f)
    denoh[:, 0:128] = 1.0
    denoh[:, dencol[1:]] = 0.0
    for j in range(1, 4):
        denoh[:, 128 * j + dencol[j]] = 1.0
    denoh = denoh.astype(NP_BF16)

    sel2v = np.zeros((128, 128), f)
    for base in (0, 64):
        sel2v[base + 0, 0:64] = 1.0
        sel2v[base + 1, 64:128] = 1.0
    sel2v = sel2v.astype(NP_BF16)

    onesb = np.ones((128, 65), NP_BF16)  # col0: LN ones

    biases = np.zeros((128, 4 * NCH), f)
    biases[:, 0:NCH] = (WS * np.asarray(sr_b, f)).reshape(NCH, 128).T
    biases[:, NCH:2 * NCH] = np.asarray(ln_g, f).reshape(NCH, 128).T
    biases[:, 2 * NCH:3 * NCH] = np.asarray(ln_b, f).reshape(NCH, 128).T
    biases[:, 3 * NCH:4 * NCH] = np.asarray(proj_b, f).reshape(NCH, 128).T

    shared = dict(wq_dr=wq_dr, wk_dr=wk_dr, wv_dr=wv_dr, srw_dr=srw_dr,
                  xi_dr=xi_dr, pj_dr=pj_dr, ck=ck2, sk=sk2, tri_dr=tri_drp,
                  denoh=denoh, sel2=sel2v, onesb=onesb, biases=biases)

    in_maps = []
    for core in range(N_CORES):
        ns = slice(core * NQ, (core + 1) * NQ)
        xs = x[:, ns, :]                                  # [2, 512, 768]
        xT = xs.transpose(2, 0, 1).reshape(C, NQT)
        xT_drp = _dr_pack(xT, 3, f8q)
        cqc = np.ascontiguousarray(
            np.concatenate([cq_full[:, ns]] * B, axis=1))
        sqc = np.ascontiguousarray(
            np.concatenate([sq_full[:, ns]] * B, axis=1))
        oh = (np.arange(M)[:, None] == (vis[ns] - 1)[None, :]).astype(f)
        oh_drp = np.ascontiguousarray(
            oh.reshape(2, 48, NQ).transpose(1, 0, 2)).astype(NP_FP8)
        in_maps.append(dict(shared, xT_dr=xT_drp, cq=cqc, sq=sqc,
                            oh_dr=oh_drp))
    return in_maps


_NC_CACHE = {}


def _get_program():
    if "nc" not in _NC_CACHE:
        _NC_CACHE["nc"] = build_program()
    return _NC_CACHE["nc"]


def kernel(x, Wq, Wkv, sr_w, sr_b, ln_g, ln_b, proj_w, proj_b, H=None, W=None,
           _trace=False):
    nc = _get_program()
    in_maps = _host_prep(x, Wq, Wkv, sr_w, sr_b, ln_g, ln_b, proj_w, proj_b)
    res = run_bass_kernel_spmd(nc, in_maps, list(range(N_CORES)),
                               trace=_trace)
    kernel.last_result = res
    out = np.empty((B, N, C), np.float32)
    for core in range(N_CORES):
        yT = np.asarray(res.results[core]["y"]).astype(np.float32)  # [768,1024]
        y = yT.reshape(C, B, NQ).transpose(1, 2, 0)       # [B, 512, 768]
        out[:, core * NQ:(core + 1) * NQ, :] = y
    return out


# revision 30
# speedup vs baseline: 1.1283x; 1.0445x over previous
"""Trainium2 Bass kernel for nn_GSAttention (spatial-reduction attention).

Strategy (v2)
-------------
* Queries sharded 512/core x 8 cores; B=2 kept on-core (1024 query rows).
* Only the first 96 reduced KV tokens are visible (max vis = 94), so the
  whole KV path (conv + LN + KV proj) runs on 96 tokens, replicated.
* All big matmuls in fp8e4m3 DoubleRow mode (4x bf16 throughput):
  q-proj, conv, k-proj, v-proj, out-proj, and the additive causal mask
  (one-hot matmul, values 0/-240 exact in fp8).  QK / AV / softmax-denom /
  denom-broadcast matmuls in bf16.  Weights are host-scaled x64 to clear
  the fp8 subnormal range; the scale is folded back into the RoPE tables
  (q/k paths) and the final output-evacuation scale (v/proj paths).
* Mask as matmul: z += tri^T @ onehot(vis) accumulated into the QK PSUM
  group; exp then masks to ~1e-13 naturally.
* Softmax denominators: per (head,batch) pair j in a group of 4, a
  [96,128] one-hot-column stationary places den_j at PSUM partition 32j;
  one DVE reciprocal per group; per-pair broadcast matmul (ones row at
  partition 32j x rec row) rebuilds [64,512] 1/den; the division fuses
  the PSUM->SBUF evacuation of AV into the DR-packed fp8 att2 tile.
* Out-proj in [c_out, q] orientation so the projection bias and the
  1/4096 fp8-scale fold into the Activation-engine PSUM evacuation.
  Host transposes the returned [768, 1024] tiles.
* One DMA per tensor, host pre-packed in device layouts (fp8/bf16):
  ~8.5 MB total HBM traffic per core.
"""

import os
import sys

for _p in ("/opt/trn_rl_repo", "/root/.axon_site/_ro/trn_rl_repo"):
    if os.path.isdir(_p) and _p not in sys.path:
        sys.path.insert(0, _p)

from contextlib import ExitStack

import numpy as np
import ml_dtypes

import types as _types
if "antenv.axon_hooks" not in sys.modules:
    _axh = _types.ModuleType("antenv.axon_hooks")
    _axh.get_axon_ntff_profile_hook = lambda: None
    sys.modules["antenv.axon_hooks"] = _axh

import concourse.bacc as bacc
import concourse.mybir as mybir
from concourse.tile import TileContext
from concourse.bass_utils import run_bass_kernel_spmd

F32 = mybir.dt.float32
BF16 = mybir.dt.bfloat16
FP8 = mybir.dt.float8e4
AF = mybir.ActivationFunctionType
ALU = mybir.AluOpType
DR = mybir.MatmulPerfMode.DoubleRow

NP_BF16 = ml_dtypes.bfloat16
NP_FP8 = ml_dtypes.float8_e4m3

# Problem constants.
N_CORES = 8
B = 2
N = 4096
C = 768
HEADS = 12
HD = 64
SR = 2
H = W = 64
NQ = 512            # queries per core per batch
NQT = B * NQ        # query rows per core
M = 96              # padded visible reduced tokens (real max vis = 94)
M2 = B * M
KC = C * SR * SR    # 3072 conv contraction
SCALE = 1.0 / 8.0
NEG = -240.0        # fp8e4m3 max; exp(scale*-240) ~ 1e-13
NCH = C // 128      # 6 feature chunks

# fp8 quantization of any data path costs 2.6-4% max-rel-err (threshold 2e-2),
# so data matmuls run bf16; only the exact-valued mask matmul uses fp8 DR.
FP8_PATHS = set()
WS = 64.0 if FP8_PATHS else 1.0


def build_program():
    nc = bacc.Bacc("TRN2", target_bir_lowering=False, debug=False,
                   num_devices=N_CORES)

    def par(name, shape, dt, out=False):
        return nc.declare_dram_parameter(name, list(shape), dt, isOutput=out)

    # host-packed inputs (one DMA each)
    DT_Q = FP8 if "q" in FP8_PATHS else BF16
    DT_CONV = FP8 if "conv" in FP8_PATHS else BF16
    DT_KV = FP8 if "kv" in FP8_PATHS else BF16
    DT_ATT = FP8 if "att" in FP8_PATHS else BF16
    xT_dr = par("xT_dr", (128, 3, 2, NQT), DT_Q)
    wq_dr = par("wq_dr", (128, 3, 2, C), DT_Q)
    wk_dr = par("wk_dr", (128, 3, 2, C), DT_KV)
    wv_dr = par("wv_dr", (128, 3, 2, C), DT_KV)
    srw_dr = par("srw_dr", (128, 12, 2, C), DT_CONV)
    xi_dr = par("xi_dr", (128, 12, 2, M2), DT_CONV)
    pj_dr = par("pj_dr", (128, 3, 2, C), DT_ATT)
    cq = par("cq", (128, NQT), BF16)
    sq = par("sq", (128, NQT), BF16)
    ck = par("ck", (128, M2), BF16)
    sk = par("sk", (128, M2), BF16)
    tri_dr = par("tri_dr", (48, 2, M), FP8)
    oh_dr = par("oh_dr", (48, 2, NQ), FP8)
    xcorr = par("xcorr", (1, 4 * C), BF16)
    denoh = par("denoh", (M, 512), BF16)
    sel2 = par("sel2", (128, 128), BF16)
    onesb = par("onesb", (128, 65), BF16)
    biases = par("biases", (128, 4 * NCH), F32)   # srb | lng | lnb | pbias
    Y = par("y", (C, NQT), BF16, out=True)

    with TileContext(nc) as tc, ExitStack() as st:
        pers = st.enter_context(tc.tile_pool(name="pers", bufs=1))

        # ---- persistent tiles ----
        cq_t = pers.tile([128, NQT], BF16, tag="cq", name="cq")
        sq_t = pers.tile([128, NQT], BF16, tag="sq", name="sq")
        ck_t = pers.tile([128, M2], BF16, tag="ck", name="ck")
        sk_t = pers.tile([128, M2], BF16, tag="sk", name="sk")
        tri_t = pers.tile([48, 2, M], FP8, tag="tri", name="tri")
        oh_t = pers.tile([48, 2, NQ], FP8, tag="oh", name="oh")
        xc_t = pers.tile([1, 4 * C], BF16, tag="xc", name="xc")
        denoh_t = pers.tile([M, 512], BF16, tag="denoh", name="denoh")
        sel2_t = pers.tile([128, 128], BF16, tag="sel2", name="sel2")
        onesb_t = pers.tile([128, 65], BF16, tag="onesb", name="onesb")
        bias_t = pers.tile([128, 4 * NCH], F32, tag="bias", name="bias")
        xT_t = pers.tile([128, 3, 2, NQT], DT_Q, tag="xT", name="xT")
        wq_t = pers.tile([128, 3, 2, C], DT_Q, tag="wq", name="wq")
        wk_t = pers.tile([128, 3, 2, C], DT_KV, tag="wk", name="wk")
        wv_t = pers.tile([128, 3, 2, C], DT_KV, tag="wv", name="wv")
        srw_t = pers.tile([128, 12, 2, C], DT_CONV, tag="srw", name="srw")
        xi_t = pers.tile([128, 12, 2, M2], DT_CONV, tag="xi", name="xi")
        pj_t = pers.tile([128, 3, 2, C], DT_ATT, tag="pj", name="pj")
        rotq = [pers.tile([128, NQT], BF16, tag=f"rotq{i}", name=f"rotq{i}")
                for i in range(NCH)]
        rotk = [pers.tile([128, M2], BF16, tag=f"rotk{i}", name=f"rotk{i}")
                for i in range(NCH)]
        vaug = [pers.tile([M, C], BF16, tag=f"vaug{b}", name=f"vaug{b}")
                for b in range(B)]
        att2 = pers.tile([128, 3, 2, NQT], DT_ATT, tag="att2", name="att2")

        # DMA priority order: q path first, then KV path, proj last.
        nc.sync.dma_start(out=xT_t[:, 0, :, 0:NQ], in_=xT_dr[:, 0, :, 0:NQ])
        nc.sync.dma_start(out=wq_t[:, 0, :, 0:128], in_=wq_dr[:, 0, :, 0:128])
        nc.sync.dma_start(out=bias_t[:], in_=biases[:])
        nc.sync.dma_start(out=wq_t[:, 0, :, 128:C], in_=wq_dr[:, 0, :, 128:C])
        nc.sync.dma_start(out=xT_t[:, 0, :, NQ:NQT], in_=xT_dr[:, 0, :, NQ:NQT])
        for j in range(1, 3):
            nc.sync.dma_start(out=xT_t[:, j, :, :], in_=xT_dr[:, j, :, :])
            nc.sync.dma_start(out=wq_t[:, j, :, :], in_=wq_dr[:, j, :, :])
        nc.sync.dma_start(out=cq_t[:], in_=cq[:])
        nc.sync.dma_start(out=sq_t[:], in_=sq[:])
        nc.sync.dma_start(out=xi_t[:], in_=xi_dr[:])
        for g in range(2):
            nc.sync.dma_start(out=srw_t[:, 6 * g:6 * (g + 1), :, :],
                              in_=srw_dr[:, 6 * g:6 * (g + 1), :, :])
        nc.sync.dma_start(out=wk_t[:], in_=wk_dr[:])
        nc.sync.dma_start(out=wv_t[:], in_=wv_dr[:])
        nc.sync.dma_start(out=ck_t[:], in_=ck[:])
        nc.sync.dma_start(out=sk_t[:], in_=sk[:])
        nc.sync.dma_start(out=tri_t[:], in_=tri_dr[:])
        nc.sync.dma_start(out=oh_t[:], in_=oh_dr[:])
        nc.sync.dma_start(out=xc_t[:], in_=xcorr[:])
        nc.sync.dma_start(out=denoh_t[:], in_=denoh[:])
        nc.sync.dma_start(out=sel2_t[:], in_=sel2[:])
        nc.sync.dma_start(out=onesb_t[:], in_=onesb[:])
        nc.sync.dma_start(out=pj_t[:], in_=pj_dr[:])

        def mm_chain(out_ap, stat_fn, mov_fn, nblk, fp8, tp=None):
            """Accumulate out += stat_j^T @ mov_j over nblk k-blocks."""
            for j in range(nblk):
                if fp8:
                    nc.tensor.matmul(out_ap, stat_fn(j, None), mov_fn(j, None),
                                     start=(j == 0), stop=(j == nblk - 1),
                                     perf_mode=DR, tile_position=tp)
                else:
                    for t in range(2):
                        nc.tensor.matmul(out_ap, stat_fn(j, t), mov_fn(j, t),
                                         start=(j == 0 and t == 0),
                                         stop=(j == nblk - 1 and t == 1),
                                         tile_position=tp)

        def dr_slice(tile, j, t, cols):
            return tile[:, j, :, cols] if t is None else tile[:, j, t, cols]

        warm = pers.tile([1, 4], F32, tag="warm", name="warm")
        nc.scalar.activation(warm[:, 0:1], bias_t[0:1, 0:1], AF.Identity)
        nc.scalar.activation(warm[:, 1:2], bias_t[0:1, 0:1], AF.Exp)
        nc.scalar.activation(warm[:, 2:3], bias_t[0:1, 0:1], AF.Sqrt,
                             bias=warm[:, 1:2])

        srb_c = lambda o: bias_t[:, o:o + 1]
        lng_c = lambda o: bias_t[:, NCH + o:NCH + o + 1]
        lnb_c = lambda o: bias_t[:, 2 * NCH + o:2 * NCH + o + 1]
        pb_c = lambda o: bias_t[:, 3 * NCH + o:3 * NCH + o + 1]

        # ========== Phase 1: q-projection + conv (fp8 DR) ==========
        with tc.tile_pool(name="p1", bufs=1) as p1, \
             tc.tile_pool(name="p1q", bufs=2) as p1q, \
             tc.tile_pool(name="ps1", bufs=1, space="PSUM") as ps1:
            q_sb = [p1.tile([128, NQT], BF16, tag=f"qsb{i}", name=f"qsb{i}")
                    for i in range(NCH)]

            def q_block(cc, half):
                ns = slice(half * NQ, (half + 1) * NQ)
                q_ps = ps1.tile([128, NQ], F32, tag=f"qp{(2 * cc + half) % 2}",
                                name=f"qp{cc}{half}")
                mm_chain(q_ps[:],
                         lambda j, t: dr_slice(wq_t, j, t,
                                               slice(cc * 128, (cc + 1) * 128)),
                         lambda j, t: dr_slice(xT_t, j, t, ns),
                         3, "q" in FP8_PATHS)
                nc.scalar.activation(q_sb[cc][:, ns], q_ps[:], AF.Identity)

            xr_ps = [ps1.tile([128, M2], F32, tag=f"xr{o}", name=f"xr{o}")
                     for o in range(NCH)]

            conv_fp8 = "conv" in FP8_PATHS

            def conv_col(o, jlo, jhi):
                for j2 in range(jlo, jhi):
                    if conv_fp8:
                        nc.tensor.matmul(
                            xr_ps[o][:], srw_t[:, j2, :, o * 128:(o + 1) * 128],
                            xi_t[:, j2, :, :], start=(j2 == 0), stop=(j2 == 11),
                            perf_mode=DR)
                    else:
                        for t in range(2):
                            nc.tensor.matmul(
                                xr_ps[o][:],
                                srw_t[:, j2, t, o * 128:(o + 1) * 128],
                                xi_t[:, j2, t, :],
                                start=(j2 == 0 and t == 0),
                                stop=(j2 == 11 and t == 1))

            # interleave: 12 q blocks with 12 conv half-columns
            for step in range(12):
                q_block(step // 2, step % 2)
                o, hcol = step % 6, step // 6
                conv_col(o, 6 * hcol, 6 * (hcol + 1))

            # ---- RoPE on q (bf16 2x on DVE; half-ops split with Pool) ----
            for o in range(3):
                t1 = p1q.tile([128, NQT], BF16, tag="rt1", name="rt1")
                t2 = p1q.tile([128, NQT], BF16, tag="rt2", name="rt2")
                nc.vector.tensor_mul(t1[:], q_sb[o][:], cq_t[:])
                nc.vector.tensor_mul(t2[:], q_sb[o + 3][:], sq_t[:])
                nc.vector.tensor_sub(rotq[o][:], t1[:], t2[:])
                t3 = p1q.tile([128, NQT], BF16, tag="rt3", name="rt3")
                t4 = p1q.tile([128, NQT], BF16, tag="rt4", name="rt4")
                nc.vector.tensor_mul(t3[:], q_sb[o + 3][:], cq_t[:])
                nc.vector.tensor_mul(t4[:], q_sb[o][:], sq_t[:])
                nc.vector.tensor_add(rotq[o + 3][:], t3[:], t4[:])

            # ========== Phase 2: LN + K/V ==========
            xr_sb = [p1.tile([128, M2], BF16, tag=f"xs{o}", name=f"xs{o}")
                     for o in range(NCH)]
            for o in range(NCH):
                nc.scalar.activation(xr_sb[o][:], xr_ps[o][:],
                                     AF.Identity, bias=srb_c(o))

            ones_ln = onesb_t[:, 0:1]
            sum_ps = ps1.tile([1, M2], F32, tag="xr0", name="sum")
            for o in range(NCH):
                nc.tensor.matmul(sum_ps[:], ones_ln, xr_sb[o][:],
                                 start=(o == 0), stop=(o == NCH - 1))
            ssq_ps = ps1.tile([1, M2], F32, tag="xr1", name="ssq")
            for o in range(NCH):
                sqt = p1q.tile([128, M2], BF16, tag="sqt", name="sqt")
                nc.gpsimd.tensor_mul(sqt[:], xr_sb[o][:], xr_sb[o][:])
                nc.tensor.matmul(ssq_ps[:], ones_ln, sqt[:],
                                 start=(o == 0), stop=(o == NCH - 1))
            mu16 = p1.tile([1, M2], BF16, tag="mu16", name="mu16")
            std16 = p1.tile([1, M2], BF16, tag="std16", name="std16")
            mu = p1.tile([1, M2], F32, tag="mu", name="mu")
            mu2 = p1.tile([1, M2], F32, tag="mu2", name="mu2")
            var = p1.tile([1, M2], F32, tag="var", name="var")
            std = p1.tile([1, M2], F32, tag="std", name="std")
            istd = p1.tile([1, M2], F32, tag="istd", name="istd")
            nc.scalar.mul(mu[:], sum_ps[:], 1.0 / C)
            nc.vector.tensor_copy(mu16[:], mu[:])
            nc.vector.tensor_mul(mu2[:], mu[:], mu[:])
            nc.vector.scalar_tensor_tensor(var[:], ssq_ps[:], 1.0 / C, mu2[:],
                                           ALU.mult, ALU.subtract)
            eps_t = p1.tile([1, 1], F32, tag="eps", name="eps")
            nc.vector.memset(eps_t[:], 1e-5 * WS * WS)
            nc.scalar.activation(std[:], var[:], AF.Sqrt, bias=eps_t[:])
            nc.vector.tensor_copy(std16[:], std[:])
            # dummy exp: pull the Exp-table load off the first-attention path
            nc.scalar.activation(warm[:, 3:4], std[0:1, 0:1], AF.Exp)
            nc.vector.reciprocal(istd[:], std[:])
            istd_b = p1.tile([128, M2], F32, tag="istd_b", name="istd_b")
            nc.gpsimd.partition_broadcast(istd_b[:], istd[:])
            # alpha = istd as a [96,1] column per batch (PE transpose)
            alsb = p1.tile([M, 2], F32, tag="alsb", name="alsb")
            idf = p1.tile([1, 1], F32, tag="idf", name="idf")
            nc.vector.memset(idf[:], 1.0)
            for b in range(B):
                al_ps = ps1.tile([M, 1], F32, tag="qp0", name=f"al{b}")
                nc.tensor.matmul(al_ps[:], istd[0:1, b * M:(b + 1) * M],
                                 idf[:], start=True, stop=True,
                                 is_transpose=True)
                nc.vector.tensor_copy(alsb[:, b:b + 1], al_ps[:])
            # istd-scaled k rope tables
            cki = p1.tile([128, M2], BF16, tag="cki", name="cki")
            ski = p1.tile([128, M2], BF16, tag="ski", name="ski")
            nc.vector.tensor_mul(cki[:], ck_t[:], istd_b[:])
            nc.vector.tensor_mul(ski[:], sk_t[:], istd_b[:])

            # ---- K projection on raw xr (LN folded) + RoPE ----
            k_sb = [p1.tile([128, M2], BF16, tag=f"ks{o}", name=f"ks{o}")
                    for o in range(NCH)]
            for o in range(NCH):
                k_ps = ps1.tile([128, M2], F32, tag=f"qp{o % 2}", name=f"k{o}")
                for cc in range(NCH):
                    nc.tensor.matmul(k_ps[:],
                                     wk_t[:, cc // 2, cc % 2,
                                          o * 128:(o + 1) * 128],
                                     xr_sb[cc][:], start=(cc == 0), stop=False)
                nc.tensor.matmul(k_ps[:], xc_t[:, o * 128:(o + 1) * 128],
                                 mu16[:], start=False, stop=False)
                nc.tensor.matmul(k_ps[:], xc_t[:, C + o * 128:C + (o + 1) * 128],
                                 std16[:], start=False, stop=True)
                nc.scalar.activation(k_sb[o][:], k_ps[:], AF.Identity)
            for o in range(3):
                t1 = p1q.tile([128, M2], BF16, tag="kt1", name="kt1")
                t2 = p1q.tile([128, M2], BF16, tag="kt2", name="kt2")
                nc.vector.tensor_mul(t1[:], k_sb[o][:], cki[:])
                nc.vector.tensor_mul(t2[:], k_sb[o + 3][:], ski[:])
                nc.vector.tensor_sub(rotk[o][:], t1[:], t2[:])
                t3 = p1q.tile([128, M2], BF16, tag="kt3", name="kt3")
                t4 = p1q.tile([128, M2], BF16, tag="kt4", name="kt4")
                nc.vector.tensor_mul(t3[:], k_sb[o + 3][:], cki[:])
                nc.vector.tensor_mul(t4[:], k_sb[o][:], ski[:])
                nc.vector.tensor_add(rotk[o + 3][:], t3[:], t4[:])

            # ---- V projection on raw xr (LN folded via alpha scale) ----
            for b in range(B):
                ms = slice(b * M, (b + 1) * M)
                for half in range(2):
                    v_ps = ps1.tile([M, 384], F32, tag=f"qp{half}",
                                    name=f"v{b}{half}")
                    for cc in range(NCH):
                        nc.tensor.matmul(
                            v_ps[:], xr_sb[cc][:, ms],
                            wv_t[:, cc // 2, cc % 2,
                                 half * 384:(half + 1) * 384],
                            start=(cc == 0), stop=False)
                    nc.tensor.matmul(
                        v_ps[:], mu16[:, ms],
                        xc_t[:, 2 * C + half * 384:2 * C + (half + 1) * 384],
                        start=False, stop=False)
                    nc.tensor.matmul(
                        v_ps[:], std16[:, ms],
                        xc_t[:, 3 * C + half * 384:3 * C + (half + 1) * 384],
                        start=False, stop=True)
                    nc.vector.tensor_scalar(
                        vaug[b][:, half * 384:(half + 1) * 384], v_ps[:],
                        alsb[:, b:b + 1], None, ALU.mult)

        # ========== Phase 3: attention (batch 0 fully first so the
        # batch-0 output projection overlaps batch-1 attention) ==========
        with tc.tile_pool(name="p3", bufs=1) as p3, \
             tc.tile_pool(name="ps3", bufs=1, space="PSUM") as ps3, \
             tc.tile_pool(name="p4", bufs=2) as p4, \
             tc.tile_pool(name="ps4", bufs=1, space="PSUM") as ps4:

            def oproj_half(half):
                ns = slice(half * NQ, (half + 1) * NQ)
                for oc in range(NCH):
                    y_sb = p4.tile([128, NQ], BF16, tag="y", name=f"y{oc}{half}")
                    y_ps = ps4.tile([128, NQ], F32, tag=f"yp{oc % 2}",
                                    name=f"yp{oc}{half}")
                    mm_chain(y_ps[:],
                             lambda j, t: dr_slice(
                                 pj_t, j, t, slice(oc * 128, (oc + 1) * 128)),
                             lambda j, t: dr_slice(att2, j, t, ns),
                             3, "att" in FP8_PATHS)
                    if oc % 2 == 0:
                        nc.vector.tensor_scalar(y_sb[:], y_ps[:],
                                                1.0 / (WS * WS), pb_c(oc),
                                                ALU.mult, ALU.add)
                    else:
                        nc.scalar.activation(y_sb[:], y_ps[:], AF.Identity,
                                             bias=pb_c(oc),
                                             scale=1.0 / (WS * WS))
                    nc.sync.dma_start(out=Y[oc * 128:(oc + 1) * 128, ns],
                                      in_=y_sb[:])

            pairs = [(h, b) for b in range(B) for h in range(HEADS)]
            e_sb = {}
            u_sb = {}
            for g in range(6):          # groups of 4 (h,b) pairs
                den_ps = ps3.tile([128, NQ], F32, tag="dn0",
                                  name=f"dn{g}")
                for j in range(4):
                    h, b = pairs[4 * g + j]
                    hq, rs = h // 4, slice((h % 4) * 32, (h % 4) * 32 + 32)
                    ms = slice(b * M, (b + 1) * M)
                    qs = slice(b * NQ, (b + 1) * NQ)
                    z_ps = ps3.tile([M, NQ], F32, tag=f"z{j % 2}",
                                    name=f"z{g}{j}")
                    tp = ((h % 4) * 32, 0) if h % 4 == 3 else None
                    nc.tensor.matmul(z_ps[:], rotk[hq][rs, ms],
                                     rotq[hq][rs, qs], start=True, stop=False,
                                     tile_position=tp)
                    nc.tensor.matmul(z_ps[:], rotk[hq + 3][rs, ms],
                                     rotq[hq + 3][rs, qs],
                                     start=False, stop=False,
                                     tile_position=tp)
                    nc.tensor.matmul(z_ps[:], tri_t[:], oh_t[:],
                                     start=False, stop=True, perf_mode=DR,
                                     skip_group_check=True)
                    e = p3.tile([M, NQ], BF16, tag=f"e{j}", name=f"e{g}{j}")
                    nc.scalar.activation(e[:], z_ps[:], AF.Exp, scale=SCALE)
                    e_sb[(h, b)] = e
                    nc.tensor.matmul(den_ps[:],
                                     denoh_t[:, j * 128:(j + 1) * 128],
                                     e[:], start=(j == 0), stop=(j == 3))
                    u_ps = ps3.tile([HD, NQ], F32, tag=f"u{j % 2}",
                                    name=f"u{g}{j}")
                    u_sb[j] = u_ps
                    nc.tensor.matmul(u_ps[:], vaug[b][:, h * HD:(h + 1) * HD],
                                     e[:], start=True, stop=True)
                rec = p3.tile([128, NQ], BF16, tag=f"rc{g % 2}",
                              name=f"rc{g}")
                with nc.allow_low_precision(reason="softmax recip bf16"):
                    nc.vector.reciprocal(rec[:], den_ps[:])
                # rebuild 1/den as [128, 512] (two 64-row pair blocks) and
                # evacuate to SBUF so the division has a single PSUM input
                rb_sb = []
                for half2 in range(2):
                    bc_ps = ps3.tile([128, NQ], F32, tag="bc0",
                                     name=f"bc{g}{half2}")
                    nc.tensor.matmul(bc_ps[:],
                                     sel2_t[64 * half2:64 * half2 + 2, :],
                                     rec[64 * half2:64 * half2 + 2, :],
                                     start=True, stop=True)
                    rb = p3.tile([128, NQ], BF16, tag=f"rb{half2}",
                                 name=f"rb{g}{half2}")
                    nc.scalar.activation(rb[:], bc_ps[:], AF.Identity)
                    rb_sb.append(rb)
                for j in range(4):
                    h, b = pairs[4 * g + j]
                    qs = slice(b * NQ, (b + 1) * NQ)
                    e_sb.pop((h, b))
                    u_ps = u_sb.pop(j)
                    rbrow = rb_sb[j // 2][(j % 2) * 64:(j % 2) * 64 + 64, :]
                    dst = att2[(h % 2) * 64:(h % 2) * 64 + 64,
                               h // 4, (h // 2) % 2, qs]
                    nc.vector.tensor_mul(dst, u_ps[:], rbrow)
                if g == 2:
                    oproj_half(0)
            oproj_half(1)

    nc.compile()
    return nc


# ======================= host-side preparation =======================

def _angles(dim, end, w, step=1.0, bias=0.0, theta=10000.0):
    flat = np.arange(end, dtype=np.float32)
    xp = (bias + (flat % w) * step).astype(np.float32)
    yp = (bias + (flat // w) * step).astype(np.float32)
    freqs = (1.0 / theta ** (np.arange(0, dim, 4, dtype=np.float32)[: dim // 4]
                             / dim)).astype(np.float32)
    xf = np.outer(xp, freqs)
    yf = np.outer(yp, freqs)
    return np.stack([xf, yf], axis=-1).reshape(end, -1).astype(np.float32)


def _dr_pack(mat, nblk, fp8):
    """[K, F] f32 -> [128, nblk, 2, F] with k = (2j+t)*128+p."""
    K, F = mat.shape
    assert K == nblk * 256
    out = mat.reshape(nblk, 2, 128, F).transpose(2, 0, 1, 3)
    if fp8:
        return np.ascontiguousarray(np.clip(out, -240, 240)).astype(NP_FP8)
    return np.ascontiguousarray(out).astype(NP_BF16)


def _host_prep(x, Wq, Wkv, sr_w, sr_b, ln_g, ln_b, proj_w, proj_b):
    f = np.float32
    x = np.asarray(x, f)
    Wq = np.asarray(Wq, f)
    Wkv = np.asarray(Wkv, f)
    sr_w = np.asarray(sr_w, f)
    proj_w = np.asarray(proj_w, f)

    # pair-split permutation: rows 0..383 pair-first, 384..767 pair-second
    hh = np.arange(HEADS)[:, None] * HD
    jj = np.arange(HD // 2)[None, :] * 2
    perm = np.concatenate([(hh + jj).ravel(), (hh + jj + 1).ravel()])

    f8q = "q" in FP8_PATHS
    f8kv = "kv" in FP8_PATHS
    f8cv = "conv" in FP8_PATHS
    f8at = "att" in FP8_PATHS
    lg = np.asarray(ln_g, f)
    lb = np.asarray(ln_b, f)
    wq_dr = _dr_pack(WS * Wq[perm, :].T, 3, f8q)         # [c_in, out-perm]
    Wkg = Wkv[:C][perm, :] * lg[None, :]
    Wvg = Wkv[C:] * lg[None, :]
    wk_dr = _dr_pack(WS * Wkg.T, 3, f8kv)
    wv_dr = _dr_pack(WS * Wvg.T, 3, f8kv)
    xcorrv = np.concatenate([-Wkg.sum(1), Wkv[:C][perm, :] @ lb,
                             -Wvg.sum(1), Wkv[C:] @ lb])[None, :].astype(NP_BF16)
    srw_dr = _dr_pack(WS * sr_w.reshape(C, KC).T, 12, f8cv)  # [kc, out]
    pj_dr = _dr_pack(WS * proj_w.T, 3, f8at)             # [c_att, out]

    # im2col of the first 6 image rows, both batches: [3072, 192]
    strip = x[:, :6 * W, :].reshape(B, 3, 2, 32, 2, C)   # b, i, di, j, dj, c
    xi2c = strip.transpose(5, 2, 4, 0, 1, 3).reshape(KC, M2)
    xi_dr = _dr_pack(xi2c, 12, f8cv)

    # RoPE tables (per-row freq pattern), q tables folded with 1/WS
    ang_q = _angles(HD, N, W)
    ang_k = _angles(HD, N // (SR * SR), W, step=SR, bias=1.0 - 1.0 / SR)
    rowj = np.arange(128) % 32
    cq_full = (np.cos(ang_q)[:, rowj].T / WS).astype(NP_BF16)   # [128, 4096]
    sq_full = (np.sin(ang_q)[:, rowj].T / WS).astype(NP_BF16)
    ckk = np.cos(ang_k)[:M, rowj].T / WS
    skk = np.sin(ang_k)[:M, rowj].T / WS
    ck2 = np.ascontiguousarray(np.concatenate([ckk, ckk], 1)).astype(NP_BF16)
    sk2 = np.ascontiguousarray(np.concatenate([skk, skk], 1)).astype(NP_BF16)

    # visibility
    n_all = np.arange(N)
    xpos = n_all // (SR * H)
    ox = n_all // H
    oy = n_all % H
    vis = xpos * SR + (ox + oy * H) // (SR * H) + 1       # [4096], <= 94

    # mask matmul stationary: tri[kk, m] = NEG if m >= kk+1
    tri = np.where(np.arange(M)[None, :] >= np.arange(M)[:, None] + 1,
                   NEG, 0.0).astype(f)                    # [96, 96]
    tri_drp = np.ascontiguousarray(
        tri.reshape(2, 48, M).transpose(1, 0, 2)).astype(NP_FP8)

    # den one-hot stationary: block j puts den_j at psum partition
    # {0,1,64,65}[j]; block 0 all ones except those (keeps rows finite)
    dencol = [0, 1, 64, 65]
    denoh = np.zeros((M, 512), f)
    denoh[:, 0:128] = 1.0
    denoh[:, dencol[1:]] = 0.0
    for j in range(1, 4):
        denoh[:, 128 * j + dencol[j]] = 1.0
    denoh = denoh.astype(NP_BF16)

    sel2v = np.zeros((128, 128), f)
    for base in (0, 64):
        sel2v[base + 0, 0:64] = 1.0
        sel2v[base + 1, 64:128] = 1.0
    sel2v = sel2v.astype(NP_BF16)

    onesb = np.ones((128, 65), NP_BF16)  # col0: LN ones

    biases = np.zeros((128, 4 * NCH), f)
    biases[:, 0:NCH] = (WS * np.asarray(sr_b, f)).reshape(NCH, 128).T
    biases[:, NCH:2 * NCH] = np.asarray(ln_g, f).reshape(NCH, 128).T
    biases[:, 2 * NCH:3 * NCH] = np.asarray(ln_b, f).reshape(NCH, 128).T
    biases[:, 3 * NCH:4 * NCH] = np.asarray(proj_b, f).reshape(NCH, 128).T

    shared = dict(wq_dr=wq_dr, wk_dr=wk_dr, wv_dr=wv_dr, srw_dr=srw_dr,
                  xi_dr=xi_dr, pj_dr=pj_dr, ck=ck2, sk=sk2, tri_dr=tri_drp,
                  xcorr=xcorrv,
                  denoh=denoh, sel2=sel2v, onesb=onesb, biases=biases)

    in_maps = []
    for core in range(N_CORES):
        ns = slice(core * NQ, (core + 1) * NQ)
        xs = x[:, ns, :]                                  # [2, 512, 768]
        xT = xs.transpose(2, 0, 1).reshape(C, NQT)
        xT_drp = _dr_pack(xT, 3, f8q)
        cqc = np.ascontiguousarray(
            np.concatenate([cq_full[:, ns]] * B, axis=1))
        sqc = np.ascontiguousarray(
            np.concatenate([sq_full[:, ns]] * B, axis=1))
        oh = (np.arange(M)[:, None] == (vis[ns] - 1)[None, :]).astype(f)
        oh_drp = np.ascontiguousarray(
            oh.reshape(2, 48, NQ).transpose(1, 0, 2)).astype(NP_FP8)
        in_maps.append(dict(shared, xT_dr=xT_drp, cq=cqc, sq=sqc,
                            oh_dr=oh_drp))
    return in_maps


_NC_CACHE = {}


def _get_program():
    if "nc" not in _NC_CACHE:
        _NC_CACHE["nc"] = build_program()
    return _NC_CACHE["nc"]


def kernel(x, Wq, Wkv, sr_w, sr_b, ln_g, ln_b, proj_w, proj_b, H=None, W=None,
           _trace=False):
    nc = _get_program()
    in_maps = _host_prep(x, Wq, Wkv, sr_w, sr_b, ln_g, ln_b, proj_w, proj_b)
    res = run_bass_kernel_spmd(nc, in_maps, list(range(N_CORES)),
                               trace=_trace)
    kernel.last_result = res
    out = np.empty((B, N, C), np.float32)
    for core in range(N_CORES):
        yT = np.asarray(res.results[core]["y"]).astype(np.float32)  # [768,1024]
        y = yT.reshape(C, B, NQ).transpose(1, 2, 0)       # [B, 512, 768]
        out[:, core * NQ:(core + 1) * NQ, :] = y
    return out


# revision 32
# speedup vs baseline: 1.1869x; 1.0519x over previous
"""Trainium2 Bass kernel for nn_GSAttention (spatial-reduction attention).

Strategy (v2)
-------------
* Queries sharded 512/core x 8 cores; B=2 kept on-core (1024 query rows).
* Only the first 96 reduced KV tokens are visible (max vis = 94), so the
  whole KV path (conv + LN + KV proj) runs on 96 tokens, replicated.
* All big matmuls in fp8e4m3 DoubleRow mode (4x bf16 throughput):
  q-proj, conv, k-proj, v-proj, out-proj, and the additive causal mask
  (one-hot matmul, values 0/-240 exact in fp8).  QK / AV / softmax-denom /
  denom-broadcast matmuls in bf16.  Weights are host-scaled x64 to clear
  the fp8 subnormal range; the scale is folded back into the RoPE tables
  (q/k paths) and the final output-evacuation scale (v/proj paths).
* Mask as matmul: z += tri^T @ onehot(vis) accumulated into the QK PSUM
  group; exp then masks to ~1e-13 naturally.
* Softmax denominators: per (head,batch) pair j in a group of 4, a
  [96,128] one-hot-column stationary places den_j at PSUM partition 32j;
  one DVE reciprocal per group; per-pair broadcast matmul (ones row at
  partition 32j x rec row) rebuilds [64,512] 1/den; the division fuses
  the PSUM->SBUF evacuation of AV into the DR-packed fp8 att2 tile.
* Out-proj in [c_out, q] orientation so the projection bias and the
  1/4096 fp8-scale fold into the Activation-engine PSUM evacuation.
  Host transposes the returned [768, 1024] tiles.
* One DMA per tensor, host pre-packed in device layouts (fp8/bf16):
  ~8.5 MB total HBM traffic per core.
"""

import os
import sys

for _p in ("/opt/trn_rl_repo", "/root/.axon_site/_ro/trn_rl_repo"):
    if os.path.isdir(_p) and _p not in sys.path:
        sys.path.insert(0, _p)

from contextlib import ExitStack

import numpy as np
import ml_dtypes

import types as _types
if "antenv.axon_hooks" not in sys.modules:
    _axh = _types.ModuleType("antenv.axon_hooks")
    _axh.get_axon_ntff_profile_hook = lambda: None
    sys.modules["antenv.axon_hooks"] = _axh

import concourse.bacc as bacc
import concourse.mybir as mybir
from concourse.tile import TileContext
from concourse.bass_utils import run_bass_kernel_spmd

F32 = mybir.dt.float32
BF16 = mybir.dt.bfloat16
FP8 = mybir.dt.float8e4
AF = mybir.ActivationFunctionType
ALU = mybir.AluOpType
DR = mybir.MatmulPerfMode.DoubleRow

NP_BF16 = ml_dtypes.bfloat16
NP_FP8 = ml_dtypes.float8_e4m3

# Problem constants.
N_CORES = 8
B = 2
N = 4096
C = 768
HEADS = 12
HD = 64
SR = 2
H = W = 64
NQ = 512            # queries per core per batch
NQT = B * NQ        # query rows per core
M = 96              # padded visible reduced tokens (real max vis = 94)
M2 = B * M
KC = C * SR * SR    # 3072 conv contraction
SCALE = 1.0 / 8.0
NEG = -240.0        # fp8e4m3 max; exp(scale*-240) ~ 1e-13
NCH = C // 128      # 6 feature chunks

# fp8 quantization of any data path costs 2.6-4% max-rel-err (threshold 2e-2),
# so data matmuls run bf16; only the exact-valued mask matmul uses fp8 DR.
FP8_PATHS = set()
WS = 64.0 if FP8_PATHS else 1.0


def build_program():
    nc = bacc.Bacc("TRN2", target_bir_lowering=False, debug=False,
                   num_devices=N_CORES)

    def par(name, shape, dt, out=False):
        return nc.declare_dram_parameter(name, list(shape), dt, isOutput=out)

    # host-packed inputs (one DMA each)
    DT_Q = FP8 if "q" in FP8_PATHS else BF16
    DT_CONV = FP8 if "conv" in FP8_PATHS else BF16
    DT_KV = FP8 if "kv" in FP8_PATHS else BF16
    DT_ATT = FP8 if "att" in FP8_PATHS else BF16
    xT_dr = par("xT_dr", (128, 3, 2, NQT), DT_Q)
    wq_dr = par("wq_dr", (128, 3, 2, C), DT_Q)
    wk_dr = par("wk_dr", (128, 3, 2, C), DT_KV)
    wv_dr = par("wv_dr", (128, 3, 2, C), DT_KV)
    srw_dr = par("srw_dr", (128, 12, 2, C), DT_CONV)
    xi_dr = par("xi_dr", (128, 12, 2, M2), DT_CONV)
    pj_dr = par("pj_dr", (128, 3, 2, C), DT_ATT)
    cq = par("cq", (128, NQT), BF16)
    sq = par("sq", (128, NQT), BF16)
    ck = par("ck", (128, M2), BF16)
    sk = par("sk", (128, M2), BF16)
    tri_dr = par("tri_dr", (48, 2, M), FP8)
    oh_dr = par("oh_dr", (48, 2, NQ), FP8)
    xcorr = par("xcorr", (1, 4 * C), BF16)
    denoh = par("denoh", (M, 512), BF16)
    sel2 = par("sel2", (128, 128), BF16)
    onesb = par("onesb", (128, 65), BF16)
    biases = par("biases", (128, 4 * NCH), F32)   # srb | lng | lnb | pbias
    Y = par("y", (C, NQT), BF16, out=True)

    with TileContext(nc) as tc, ExitStack() as st:
        pers = st.enter_context(tc.tile_pool(name="pers", bufs=1))

        # ---- persistent tiles ----
        cq_t = pers.tile([128, NQT], BF16, tag="cq", name="cq")
        sq_t = pers.tile([128, NQT], BF16, tag="sq", name="sq")
        ck_t = pers.tile([128, M2], BF16, tag="ck", name="ck")
        sk_t = pers.tile([128, M2], BF16, tag="sk", name="sk")
        tri_t = pers.tile([48, 2, M], FP8, tag="tri", name="tri")
        oh_t = pers.tile([48, 2, NQ], FP8, tag="oh", name="oh")
        xc_t = pers.tile([1, 4 * C], BF16, tag="xc", name="xc")
        denoh_t = pers.tile([M, 512], BF16, tag="denoh", name="denoh")
        sel2_t = pers.tile([128, 128], BF16, tag="sel2", name="sel2")
        onesb_t = pers.tile([128, 65], BF16, tag="onesb", name="onesb")
        bias_t = pers.tile([128, 4 * NCH], F32, tag="bias", name="bias")
        xT_t = pers.tile([128, 3, 2, NQT], DT_Q, tag="xT", name="xT")
        wq_t = pers.tile([128, 3, 2, C], DT_Q, tag="wq", name="wq")
        wk_t = pers.tile([128, 3, 2, C], DT_KV, tag="wk", name="wk")
        wv_t = pers.tile([128, 3, 2, C], DT_KV, tag="wv", name="wv")
        srw_t = pers.tile([128, 12, 2, C], DT_CONV, tag="srw", name="srw")
        xi_t = pers.tile([128, 12, 2, M2], DT_CONV, tag="xi", name="xi")
        pj_t = pers.tile([128, 3, 2, C], DT_ATT, tag="pj", name="pj")
        rotq = [pers.tile([128, NQT], BF16, tag=f"rotq{i}", name=f"rotq{i}")
                for i in range(NCH)]
        rotk = [pers.tile([128, M2], BF16, tag=f"rotk{i}", name=f"rotk{i}")
                for i in range(NCH)]
        vaug = [pers.tile([M, C], BF16, tag=f"vaug{b}", name=f"vaug{b}")
                for b in range(B)]
        att2 = pers.tile([128, 3, 2, NQT], DT_ATT, tag="att2", name="att2")

        # DMA priority order: q path first, then KV path, proj last.
        nc.sync.dma_start(out=xT_t[:, 0, :, 0:NQ], in_=xT_dr[:, 0, :, 0:NQ])
        nc.sync.dma_start(out=wq_t[:, 0, :, 0:128], in_=wq_dr[:, 0, :, 0:128])
        nc.sync.dma_start(out=bias_t[:], in_=biases[:])
        nc.sync.dma_start(out=wq_t[:, 0, :, 128:C], in_=wq_dr[:, 0, :, 128:C])
        nc.sync.dma_start(out=xT_t[:, 0, :, NQ:NQT], in_=xT_dr[:, 0, :, NQ:NQT])
        for j in range(1, 3):
            nc.sync.dma_start(out=xT_t[:, j, :, :], in_=xT_dr[:, j, :, :])
            nc.sync.dma_start(out=wq_t[:, j, :, :], in_=wq_dr[:, j, :, :])
        nc.sync.dma_start(out=cq_t[:], in_=cq[:])
        nc.sync.dma_start(out=sq_t[:], in_=sq[:])
        nc.sync.dma_start(out=xi_t[:], in_=xi_dr[:])
        for g in range(2):
            nc.sync.dma_start(out=srw_t[:, 6 * g:6 * (g + 1), :, :],
                              in_=srw_dr[:, 6 * g:6 * (g + 1), :, :])
        nc.sync.dma_start(out=wk_t[:], in_=wk_dr[:])
        nc.sync.dma_start(out=wv_t[:], in_=wv_dr[:])
        nc.sync.dma_start(out=ck_t[:], in_=ck[:])
        nc.sync.dma_start(out=sk_t[:], in_=sk[:])
        nc.sync.dma_start(out=tri_t[:], in_=tri_dr[:])
        nc.sync.dma_start(out=oh_t[:], in_=oh_dr[:])
        nc.sync.dma_start(out=xc_t[:], in_=xcorr[:])
        nc.sync.dma_start(out=denoh_t[:], in_=denoh[:])
        nc.sync.dma_start(out=sel2_t[:], in_=sel2[:])
        nc.sync.dma_start(out=onesb_t[:], in_=onesb[:])
        nc.sync.dma_start(out=pj_t[:], in_=pj_dr[:])

        def mm_chain(out_ap, stat_fn, mov_fn, nblk, fp8, tp=None):
            """Accumulate out += stat_j^T @ mov_j over nblk k-blocks."""
            for j in range(nblk):
                if fp8:
                    nc.tensor.matmul(out_ap, stat_fn(j, None), mov_fn(j, None),
                                     start=(j == 0), stop=(j == nblk - 1),
                                     perf_mode=DR, tile_position=tp)
                else:
                    for t in range(2):
                        nc.tensor.matmul(out_ap, stat_fn(j, t), mov_fn(j, t),
                                         start=(j == 0 and t == 0),
                                         stop=(j == nblk - 1 and t == 1),
                                         tile_position=tp)

        def dr_slice(tile, j, t, cols):
            return tile[:, j, :, cols] if t is None else tile[:, j, t, cols]

        warm = pers.tile([1, 4], F32, tag="warm", name="warm")
        nc.scalar.activation(warm[:, 0:1], bias_t[0:1, 0:1], AF.Identity)
        nc.scalar.activation(warm[:, 1:2], bias_t[0:1, 0:1], AF.Exp)
        nc.scalar.activation(warm[:, 2:3], bias_t[0:1, 0:1], AF.Sqrt,
                             bias=warm[:, 1:2])

        srb_c = lambda o: bias_t[:, o:o + 1]
        lng_c = lambda o: bias_t[:, NCH + o:NCH + o + 1]
        lnb_c = lambda o: bias_t[:, 2 * NCH + o:2 * NCH + o + 1]
        pb_c = lambda o: bias_t[:, 3 * NCH + o:3 * NCH + o + 1]

        # ========== Phase 1: q-projection + conv (fp8 DR) ==========
        with tc.tile_pool(name="p1", bufs=1) as p1, \
             tc.tile_pool(name="p1q", bufs=2) as p1q, \
             tc.tile_pool(name="ps1", bufs=1, space="PSUM") as ps1:
            q_sb = [p1.tile([128, NQT], BF16, tag=f"qsb{i}", name=f"qsb{i}")
                    for i in range(NCH)]

            def q_block(cc, half):
                ns = slice(half * NQ, (half + 1) * NQ)
                q_ps = ps1.tile([128, NQ], F32, tag=f"qp{(2 * cc + half) % 2}",
                                name=f"qp{cc}{half}")
                mm_chain(q_ps[:],
                         lambda j, t: dr_slice(wq_t, j, t,
                                               slice(cc * 128, (cc + 1) * 128)),
                         lambda j, t: dr_slice(xT_t, j, t, ns),
                         3, "q" in FP8_PATHS)
                nc.scalar.activation(q_sb[cc][:, ns], q_ps[:], AF.Identity)

            xr_ps = [ps1.tile([128, M2], F32, tag=f"xr{o}", name=f"xr{o}")
                     for o in range(NCH)]

            conv_fp8 = "conv" in FP8_PATHS

            def conv_col(o, jlo, jhi):
                for j2 in range(jlo, jhi):
                    if conv_fp8:
                        nc.tensor.matmul(
                            xr_ps[o][:], srw_t[:, j2, :, o * 128:(o + 1) * 128],
                            xi_t[:, j2, :, :], start=(j2 == 0), stop=(j2 == 11),
                            perf_mode=DR)
                    else:
                        for t in range(2):
                            nc.tensor.matmul(
                                xr_ps[o][:],
                                srw_t[:, j2, t, o * 128:(o + 1) * 128],
                                xi_t[:, j2, t, :],
                                start=(j2 == 0 and t == 0),
                                stop=(j2 == 11 and t == 1))

            # interleave q blocks, conv half-columns, and rope pairs
            def rope_pair(o):
                t1 = p1q.tile([128, NQT], BF16, tag="rt1", name="rt1")
                t2 = p1q.tile([128, NQT], BF16, tag="rt2", name="rt2")
                nc.vector.tensor_mul(t1[:], q_sb[o][:], cq_t[:])
                nc.vector.tensor_mul(t2[:], q_sb[o + 3][:], sq_t[:])
                nc.vector.tensor_sub(rotq[o][:], t1[:], t2[:])
                t3 = p1q.tile([128, NQT], BF16, tag="rt3", name="rt3")
                t4 = p1q.tile([128, NQT], BF16, tag="rt4", name="rt4")
                nc.vector.tensor_mul(t3[:], q_sb[o + 3][:], cq_t[:])
                nc.vector.tensor_mul(t4[:], q_sb[o][:], sq_t[:])
                nc.vector.tensor_add(rotq[o + 3][:], t3[:], t4[:])

            cc_order = [0, 3, 1, 4, 2, 5]
            step = 0
            for i, cc in enumerate(cc_order):
                for half in range(2):
                    q_block(cc, half)
                    o, hcol = step % 6, step // 6
                    conv_col(o, 6 * hcol, 6 * (hcol + 1))
                    step += 1
                if i % 2 == 1:
                    rope_pair(i // 2)

            # ========== Phase 2: LN + K/V ==========
            xr_sb = [p1.tile([128, M2], BF16, tag=f"xs{o}", name=f"xs{o}")
                     for o in range(NCH)]
            for o in range(NCH):
                nc.scalar.activation(xr_sb[o][:], xr_ps[o][:],
                                     AF.Identity, bias=srb_c(o))

            ones_ln = onesb_t[:, 0:1]
            sum_ps = ps1.tile([1, M2], F32, tag="xr0", name="sum")
            for o in range(NCH):
                nc.tensor.matmul(sum_ps[:], ones_ln, xr_sb[o][:],
                                 start=(o == 0), stop=(o == NCH - 1))
            ssq_ps = ps1.tile([1, M2], F32, tag="xr1", name="ssq")
            for o in range(NCH):
                sqt = p1q.tile([128, M2], BF16, tag="sqt", name="sqt")
                nc.gpsimd.tensor_mul(sqt[:], xr_sb[o][:], xr_sb[o][:])
                nc.tensor.matmul(ssq_ps[:], ones_ln, sqt[:],
                                 start=(o == 0), stop=(o == NCH - 1))
            mu16 = p1.tile([1, M2], BF16, tag="mu16", name="mu16")
            std16 = p1.tile([1, M2], BF16, tag="std16", name="std16")
            mu = p1.tile([1, M2], F32, tag="mu", name="mu")
            mu2 = p1.tile([1, M2], F32, tag="mu2", name="mu2")
            var = p1.tile([1, M2], F32, tag="var", name="var")
            std = p1.tile([1, M2], F32, tag="std", name="std")
            istd = p1.tile([1, M2], F32, tag="istd", name="istd")
            nc.scalar.mul(mu[:], sum_ps[:], 1.0 / C)
            nc.vector.tensor_copy(mu16[:], mu[:])
            nc.vector.tensor_mul(mu2[:], mu[:], mu[:])
            nc.vector.scalar_tensor_tensor(var[:], ssq_ps[:], 1.0 / C, mu2[:],
                                           ALU.mult, ALU.subtract)
            eps_t = p1.tile([1, 1], F32, tag="eps", name="eps")
            nc.vector.memset(eps_t[:], 1e-5 * WS * WS)
            nc.scalar.activation(std[:], var[:], AF.Sqrt, bias=eps_t[:])
            nc.vector.tensor_copy(std16[:], std[:])
            # dummy exp: pull the Exp-table load off the first-attention path
            nc.scalar.activation(warm[:, 3:4], std[0:1, 0:1], AF.Exp)
            nc.vector.reciprocal(istd[:], std[:])
            istd_b = p1.tile([128, M2], F32, tag="istd_b", name="istd_b")
            nc.gpsimd.partition_broadcast(istd_b[:], istd[:])
            # alpha = istd as a [96,1] column per batch (PE transpose)
            alsb = p1.tile([M, 2], F32, tag="alsb", name="alsb")
            idf = p1.tile([1, 1], F32, tag="idf", name="idf")
            nc.vector.memset(idf[:], 1.0)
            for b in range(B):
                al_ps = ps1.tile([M, 1], F32, tag="qp0", name=f"al{b}")
                nc.tensor.matmul(al_ps[:], istd[0:1, b * M:(b + 1) * M],
                                 idf[:], start=True, stop=True,
                                 is_transpose=True)
                nc.vector.tensor_copy(alsb[:, b:b + 1], al_ps[:])
            # istd-scaled k rope tables
            cki = p1.tile([128, M2], BF16, tag="cki", name="cki")
            ski = p1.tile([128, M2], BF16, tag="ski", name="ski")
            nc.vector.tensor_mul(cki[:], ck_t[:], istd_b[:])
            nc.vector.tensor_mul(ski[:], sk_t[:], istd_b[:])

            # ---- K projection on raw xr (LN folded) + RoPE ----
            k_sb = [p1.tile([128, M2], BF16, tag=f"ks{o}", name=f"ks{o}")
                    for o in range(NCH)]
            for o in range(NCH):
                k_ps = ps1.tile([128, M2], F32, tag=f"qp{o % 2}", name=f"k{o}")
                for cc in range(NCH):
                    nc.tensor.matmul(k_ps[:],
                                     wk_t[:, cc // 2, cc % 2,
                                          o * 128:(o + 1) * 128],
                                     xr_sb[cc][:], start=(cc == 0), stop=False)
                nc.tensor.matmul(k_ps[:], xc_t[:, o * 128:(o + 1) * 128],
                                 mu16[:], start=False, stop=False)
                nc.tensor.matmul(k_ps[:], xc_t[:, C + o * 128:C + (o + 1) * 128],
                                 std16[:], start=False, stop=True)
                nc.scalar.activation(k_sb[o][:], k_ps[:], AF.Identity)
            for o in range(3):
                t1 = p1q.tile([128, M2], BF16, tag="kt1", name="kt1")
                t2 = p1q.tile([128, M2], BF16, tag="kt2", name="kt2")
                nc.vector.tensor_mul(t1[:], k_sb[o][:], cki[:])
                nc.vector.tensor_mul(t2[:], k_sb[o + 3][:], ski[:])
                nc.vector.tensor_sub(rotk[o][:], t1[:], t2[:])
                t3 = p1q.tile([128, M2], BF16, tag="kt3", name="kt3")
                t4 = p1q.tile([128, M2], BF16, tag="kt4", name="kt4")
                nc.vector.tensor_mul(t3[:], k_sb[o + 3][:], cki[:])
                nc.vector.tensor_mul(t4[:], k_sb[o][:], ski[:])
                nc.vector.tensor_add(rotk[o + 3][:], t3[:], t4[:])

            # ---- V projection on raw xr (LN folded via alpha scale) ----
            for b in range(B):
                ms = slice(b * M, (b + 1) * M)
                for half in range(2):
                    v_ps = ps1.tile([M, 384], F32, tag=f"xr{2 + half}",
                                    name=f"v{b}{half}")
                    for cc in range(NCH):
                        nc.tensor.matmul(
                            v_ps[:], xr_sb[cc][:, ms],
                            wv_t[:, cc // 2, cc % 2,
                                 half * 384:(half + 1) * 384],
                            start=(cc == 0), stop=False)
                    nc.tensor.matmul(
                        v_ps[:], mu16[:, ms],
                        xc_t[:, 2 * C + half * 384:2 * C + (half + 1) * 384],
                        start=False, stop=False)
                    nc.tensor.matmul(
                        v_ps[:], std16[:, ms],
                        xc_t[:, 3 * C + half * 384:3 * C + (half + 1) * 384],
                        start=False, stop=True)
                    nc.vector.tensor_scalar(
                        vaug[b][:, half * 384:(half + 1) * 384], v_ps[:],
                        alsb[:, b:b + 1], None, ALU.mult)

        # ========== Phase 3: attention (batch 0 fully first so the
        # batch-0 output projection overlaps batch-1 attention) ==========
        with tc.tile_pool(name="p3", bufs=1) as p3, \
             tc.tile_pool(name="ps3", bufs=1, space="PSUM") as ps3, \
             tc.tile_pool(name="p4", bufs=2) as p4, \
             tc.tile_pool(name="ps4", bufs=1, space="PSUM") as ps4:

            def oproj_half(half):
                ns = slice(half * NQ, (half + 1) * NQ)
                for oc in range(NCH):
                    y_sb = p4.tile([128, NQ], BF16, tag="y", name=f"y{oc}{half}")
                    y_ps = ps4.tile([128, NQ], F32, tag=f"yp{oc % 2}",
                                    name=f"yp{oc}{half}")
                    mm_chain(y_ps[:],
                             lambda j, t: dr_slice(
                                 pj_t, j, t, slice(oc * 128, (oc + 1) * 128)),
                             lambda j, t: dr_slice(att2, j, t, ns),
                             3, "att" in FP8_PATHS)
                    if oc % 2 == 0:
                        nc.vector.tensor_scalar(y_sb[:], y_ps[:],
                                                1.0 / (WS * WS), pb_c(oc),
                                                ALU.mult, ALU.add)
                    else:
                        nc.scalar.activation(y_sb[:], y_ps[:], AF.Identity,
                                             bias=pb_c(oc),
                                             scale=1.0 / (WS * WS))
                    nc.sync.dma_start(out=Y[oc * 128:(oc + 1) * 128, ns],
                                      in_=y_sb[:])

            pairs = [(h, b) for b in range(B) for h in range(HEADS)]
            e_sb = {}
            u_sb = {}
            for g in range(6):          # groups of 4 (h,b) pairs
                den_ps = ps3.tile([128, NQ], F32, tag="dnbc",
                                  name=f"dn{g}")
                for j in range(4):
                    h, b = pairs[4 * g + j]
                    hq, rs = h // 4, slice((h % 4) * 32, (h % 4) * 32 + 32)
                    ms = slice(b * M, (b + 1) * M)
                    qs = slice(b * NQ, (b + 1) * NQ)
                    z_ps = ps3.tile([M, NQ], F32, tag=f"z{j % 3}",
                                    name=f"z{g}{j}")
                    tp = ((h % 4) * 32, 0) if h % 4 == 3 else None
                    nc.tensor.matmul(z_ps[:], rotk[hq][rs, ms],
                                     rotq[hq][rs, qs], start=True, stop=False,
                                     tile_position=tp)
                    nc.tensor.matmul(z_ps[:], rotk[hq + 3][rs, ms],
                                     rotq[hq + 3][rs, qs],
                                     start=False, stop=False,
                                     tile_position=tp)
                    nc.tensor.matmul(z_ps[:], tri_t[:], oh_t[:],
                                     start=False, stop=True, perf_mode=DR,
                                     skip_group_check=True)
                    e = p3.tile([M, NQ], BF16, tag=f"e{j}", name=f"e{g}{j}")
                    nc.scalar.activation(e[:], z_ps[:], AF.Exp, scale=SCALE)
                    e_sb[(h, b)] = e
                    nc.tensor.matmul(den_ps[:],
                                     denoh_t[:, j * 128:(j + 1) * 128],
                                     e[:], start=(j == 0), stop=(j == 3))
                    if j % 2 == 0:
                        u_sb[j // 2] = ps3.tile([128, NQ], F32,
                                                tag=f"u{(j // 2) % 2}",
                                                name=f"u{g}{j // 2}")
                    upart = u_sb[j // 2][(j % 2) * 64:(j % 2) * 64 + 64, :]
                    nc.tensor.matmul(upart, vaug[b][:, h * HD:(h + 1) * HD],
                                     e[:], start=True, stop=True,
                                     tile_position=(0, (j % 2) * 64))
                rec = p3.tile([128, NQ], BF16, tag=f"rc{g % 2}",
                              name=f"rc{g}")
                with nc.allow_low_precision(reason="softmax recip bf16"):
                    nc.vector.reciprocal(rec[:], den_ps[:])
                # rebuild 1/den as [128, 512] (two 64-row pair blocks) and
                # evacuate to SBUF so the division has a single PSUM input
                rb_sb = []
                for half2 in range(2):
                    bc_ps = ps3.tile([128, NQ], F32, tag="dnbc",
                                     name=f"bc{g}{half2}")
                    nc.tensor.matmul(bc_ps[:],
                                     sel2_t[64 * half2:64 * half2 + 2, :],
                                     rec[64 * half2:64 * half2 + 2, :],
                                     start=True, stop=True)
                    rb = p3.tile([128, NQ], BF16, tag=f"rb{half2}",
                                 name=f"rb{g}{half2}")
                    nc.scalar.activation(rb[:], bc_ps[:], AF.Identity)
                    rb_sb.append(rb)
                for jj in range(2):
                    h, b = pairs[4 * g + 2 * jj]
                    qs = slice(b * NQ, (b + 1) * NQ)
                    e_sb.pop((h, b))
                    e_sb.pop(pairs[4 * g + 2 * jj + 1])
                    u_ps = u_sb.pop(jj)
                    dst = att2[:, h // 4, (h // 2) % 2, qs]
                    nc.vector.tensor_mul(dst, u_ps[:], rb_sb[jj][:])
                if g == 2:
                    oproj_half(0)
            oproj_half(1)

    nc.compile()
    return nc


# ======================= host-side preparation =======================

def _angles(dim, end, w, step=1.0, bias=0.0, theta=10000.0):
    flat = np.arange(end, dtype=np.float32)
    xp = (bias + (flat % w) * step).astype(np.float32)
    yp = (bias + (flat // w) * step).astype(np.float32)
    freqs = (1.0 / theta ** (np.arange(0, dim, 4, dtype=np.float32)[: dim // 4]
                             / dim)).astype(np.float32)
    xf = np.outer(xp, freqs)
    yf = np.outer(yp, freqs)
    return np.stack([xf, yf], axis=-1).reshape(end, -1).astype(np.float32)


def _dr_pack(mat, nblk, fp8):
    """[K, F] f32 -> [128, nblk, 2, F] with k = (2j+t)*128+p."""
    K, F = mat.shape
    assert K == nblk * 256
    out = mat.reshape(nblk, 2, 128, F).transpose(2, 0, 1, 3)
    if fp8:
        return np.ascontiguousarray(np.clip(out, -240, 240)).astype(NP_FP8)
    return np.ascontiguousarray(out).astype(NP_BF16)


def _host_prep(x, Wq, Wkv, sr_w, sr_b, ln_g, ln_b, proj_w, proj_b):
    f = np.float32
    x = np.asarray(x, f)
    Wq = np.asarray(Wq, f)
    Wkv = np.asarray(Wkv, f)
    sr_w = np.asarray(sr_w, f)
    proj_w = np.asarray(proj_w, f)

    # pair-split permutation: rows 0..383 pair-first, 384..767 pair-second
    hh = np.arange(HEADS)[:, None] * HD
    jj = np.arange(HD // 2)[None, :] * 2
    perm = np.concatenate([(hh + jj).ravel(), (hh + jj + 1).ravel()])

    f8q = "q" in FP8_PATHS
    f8kv = "kv" in FP8_PATHS
    f8cv = "conv" in FP8_PATHS
    f8at = "att" in FP8_PATHS
    lg = np.asarray(ln_g, f)
    lb = np.asarray(ln_b, f)
    wq_dr = _dr_pack(WS * Wq[perm, :].T, 3, f8q)         # [c_in, out-perm]
    Wkg = Wkv[:C][perm, :] * lg[None, :]
    Wvg = Wkv[C:] * lg[None, :]
    wk_dr = _dr_pack(WS * Wkg.T, 3, f8kv)
    wv_dr = _dr_pack(WS * Wvg.T, 3, f8kv)
    xcorrv = np.concatenate([-Wkg.sum(1), Wkv[:C][perm, :] @ lb,
                             -Wvg.sum(1), Wkv[C:] @ lb])[None, :].astype(NP_BF16)
    srw_dr = _dr_pack(WS * sr_w.reshape(C, KC).T, 12, f8cv)  # [kc, out]
    pj_dr = _dr_pack(WS * proj_w.T, 3, f8at)             # [c_att, out]

    # im2col of the first 6 image rows, both batches: [3072, 192]
    strip = x[:, :6 * W, :].reshape(B, 3, 2, 32, 2, C)   # b, i, di, j, dj, c
    xi2c = strip.transpose(5, 2, 4, 0, 1, 3).reshape(KC, M2)
    xi_dr = _dr_pack(xi2c, 12, f8cv)

    # RoPE tables (per-row freq pattern), q tables folded with 1/WS
    ang_q = _angles(HD, N, W)
    ang_k = _angles(HD, N // (SR * SR), W, step=SR, bias=1.0 - 1.0 / SR)
    rowj = np.arange(128) % 32
    cq_full = (np.cos(ang_q)[:, rowj].T / WS).astype(NP_BF16)   # [128, 4096]
    sq_full = (np.sin(ang_q)[:, rowj].T / WS).astype(NP_BF16)
    ckk = np.cos(ang_k)[:M, rowj].T / WS
    skk = np.sin(ang_k)[:M, rowj].T / WS
    ck2 = np.ascontiguousarray(np.concatenate([ckk, ckk], 1)).astype(NP_BF16)
    sk2 = np.ascontiguousarray(np.concatenate([skk, skk], 1)).astype(NP_BF16)

    # visibility
    n_all = np.arange(N)
    xpos = n_all // (SR * H)
    ox = n_all // H
    oy = n_all % H
    vis = xpos * SR + (ox + oy * H) // (SR * H) + 1       # [4096], <= 94

    # mask matmul stationary: tri[kk, m] = NEG if m >= kk+1
    tri = np.where(np.arange(M)[None, :] >= np.arange(M)[:, None] + 1,
                   NEG, 0.0).astype(f)                    # [96, 96]
    tri_drp = np.ascontiguousarray(
        tri.reshape(2, 48, M).transpose(1, 0, 2)).astype(NP_FP8)

    # den one-hot stationary: block j puts den_j at psum partition
    # {0,1,64,65}[j]; block 0 all ones except those (keeps rows finite)
    dencol = [0, 1, 64, 65]
    denoh = np.zeros((M, 512), f)
    denoh[:, 0:128] = 1.0
    denoh[:, dencol[1:]] = 0.0
    for j in range(1, 4):
        denoh[:, 128 * j + dencol[j]] = 1.0
    denoh = denoh.astype(NP_BF16)

    sel2v = np.zeros((128, 128), f)
    for base in (0, 64):
        sel2v[base + 0, 0:64] = 1.0
        sel2v[base + 1, 64:128] = 1.0
    sel2v = sel2v.astype(NP_BF16)

    onesb = np.ones((128, 65), NP_BF16)  # col0: LN ones

    biases = np.zeros((128, 4 * NCH), f)
    biases[:, 0:NCH] = (WS * np.asarray(sr_b, f)).reshape(NCH, 128).T
    biases[:, NCH:2 * NCH] = np.asarray(ln_g, f).reshape(NCH, 128).T
    biases[:, 2 * NCH:3 * NCH] = np.asarray(ln_b, f).reshape(NCH, 128).T
    biases[:, 3 * NCH:4 * NCH] = np.asarray(proj_b, f).reshape(NCH, 128).T

    shared = dict(wq_dr=wq_dr, wk_dr=wk_dr, wv_dr=wv_dr, srw_dr=srw_dr,
                  xi_dr=xi_dr, pj_dr=pj_dr, ck=ck2, sk=sk2, tri_dr=tri_drp,
                  xcorr=xcorrv,
                  denoh=denoh, sel2=sel2v, onesb=onesb, biases=biases)

    in_maps = []
    for core in range(N_CORES):
        ns = slice(core * NQ, (core + 1) * NQ)
        xs = x[:, ns, :]                                  # [2, 512, 768]
        xT = xs.transpose(2, 0, 1).reshape(C, NQT)
        xT_drp = _dr_pack(xT, 3, f8q)
        cqc = np.ascontiguousarray(
            np.concatenate([cq_full[:, ns]] * B, axis=1))
        sqc = np.ascontiguousarray(
            np.concatenate([sq_full[:, ns]] * B, axis=1))
        oh = (np.arange(M)[:, None] == (vis[ns] - 1)[None, :]).astype(f)
        oh_drp = np.ascontiguousarray(
            oh.reshape(2, 48, NQ).transpose(1, 0, 2)).astype(NP_FP8)
        in_maps.append(dict(shared, xT_dr=xT_drp, cq=cqc, sq=sqc,
                            oh_dr=oh_drp))
    return in_maps


_NC_CACHE = {}


def _get_program():
    if "nc" not in _NC_CACHE:
        _NC_CACHE["nc"] = build_program()
    return _NC_CACHE["nc"]


def kernel(x, Wq, Wkv, sr_w, sr_b, ln_g, ln_b, proj_w, proj_b, H=None, W=None,
           _trace=False):
    nc = _get_program()
    in_maps = _host_prep(x, Wq, Wkv, sr_w, sr_b, ln_g, ln_b, proj_w, proj_b)
    res = run_bass_kernel_spmd(nc, in_maps, list(range(N_CORES)),
                               trace=_trace)
    kernel.last_result = res
    out = np.empty((B, N, C), np.float32)
    for core in range(N_CORES):
        yT = np.asarray(res.results[core]["y"]).astype(np.float32)  # [768,1024]
        y = yT.reshape(C, B, NQ).transpose(1, 2, 0)       # [B, 512, 768]
        out[:, core * NQ:(core + 1) * NQ, :] = y
    return out
